# revision 1
# baseline (speedup 1.0000x reference)
"""Causal self-attention (B=2, T=2048, C=1024, NH=16, HD=64) on 8 NeuronCores.

Sharding: core c -> (batch b = c//4, head-group hg = c%4 of 4 heads).
Each core computes the qkv projection for its 4 heads from x[b], attention
for its 4 (b,h) units, and a partial output projection (row-parallel over
the head dim). Unshard = sum of the 4 partials per batch + bproj (host).

v2 design (all matmul operands bf16, PSUM accumulation f32):
  - Host pre-transposes x and pre-swizzles every weight into the exact
    SBUF layout, so DMAs are plain [128, W] copies and the device does no
    layout work at all (the old on-PE x-transpose stage is gone).
  - B: qkT [512, T] = wqk.T @ xT + bqk, evicted per quarter as bf16.
  - C: v_aug [T, 260] = [x | 1] @ wv_aug; per head 64 v columns + a ones
    column so the softmax row-sums fall out of the AV matmul for free.
  - D: att^T[j,i] per (head, i-chunk of 512) on PE (K=64); causal mask is
    a constant [128,128] lower-triangle(-8000) tile ACCUMULATED into the
    diagonal att psum before exp (no vector-engine masking). exp on ACT
    (scale 1/8, no max subtraction -- logits are O(1) by construction),
    output bf16.
  - AV flipped: y[i-tile, 4*65] accumulated as e_block.T @ v (e is the
    stationary operand), so each j-tile costs 65 moving rows instead of
    128+. Row 64 of each head's 65-col group = softmax sum S (from the
    ones column). Normalize with per-partition scalars (reciprocal + DVE
    tensor_scalar), transpose y on PE, project: out = yT.T @ wp (bf16
    partial, no bias; host adds bproj once).
  - The emission order software-pipelines ACT(exp) against PE: attention
    stages for quarter ic interleave with B/C work of quarter ic+1 and
    the transpose+projection of quarter ic-1 via a filler queue.
"""
import os
import sys

import numpy as np

for _p in ("/opt/trn_rl_repo",):
    if _p not in sys.path and os.path.isdir(_p):
        sys.path.insert(0, _p)

import concourse.bass as bass
import concourse.mybir as mybir
import concourse.tile as tile
from concourse.masks import make_identity

B, T, C, NH, HD = 2, 2048, 1024, 16, 64
F32 = mybir.dt.float32
BF16 = mybir.dt.bfloat16
N_CORES = 8
NQ = 4          # token quarters (512 tokens each)
NKT = C // 128  # 8 contraction tiles
NT = T // 128   # 16 token tiles

# cost-model estimates (ns) used only to balance the filler interleave
MM_NS = 512 * 0.4167          # 512-row bf16 matmul
EXP_INIT_NS = 160.0           # per-exp-instruction access overhead
EXP_PAIR_NS = 1024 * 0.833 + EXP_INIT_NS


# measured-feedback filler pops (stage_seq_id -> ns), from iterating
# TimelineSim: simulate, map PE stalls to emission points, re-pop there.
_STAGE_HINTS = {79: 3000.0, 2: 754.4, 30: 121.6, 36: 121.6, 42: 345.6, 45: 396.0, 47: 296.8, 46: 354.4, 39: 504.8, 51: 333.6, 52: 216.0, 55: 145.6, 56: 197.6, 60: 355.2, 62: 144.0, 67: 398.4, 69: 144.0, 70: 268.8, 75: 629.6, 76: 446.4, 77: 446.4, 78: 757.6}


def build_nc(split_waits=True, hints=None):
    # hints: {stage_seq_id: extra_filler_ns} -- measured-feedback pops
    hints = _STAGE_HINTS if hints is None else hints
    stage_registry = []   # (stage_seq_id, first_inst_num) in emission order
    nc = bass.Bass()
    xt_d = nc.declare_dram_parameter("xt", [NQ * 128, 4096], BF16, isOutput=False)
    wqk_d = nc.declare_dram_parameter("wqk", [128, 4096], BF16, isOutput=False)
    bqk_d = nc.declare_dram_parameter("bqk", [128, 4], F32, isOutput=False)
    wv_d = nc.declare_dram_parameter("wv", [128, 8 * 260], BF16, isOutput=False)
    wvl_d = nc.declare_dram_parameter("wvl", [1, 260], BF16, isOutput=False)
    wp_d = nc.declare_dram_parameter("wp", [128, 2048], BF16, isOutput=False)
    out_d = nc.declare_dram_parameter("out", [T, C], BF16, isOutput=True)

    with tile.TileContext(nc) as tc:
        with (
            tc.tile_pool(name="const", bufs=1) as const,
            tc.tile_pool(name="wts", bufs=1) as wts,
            tc.tile_pool(name="xtp", bufs=1) as xtp,
            tc.tile_pool(name="qkt", bufs=1) as qkt,
            tc.tile_pool(name="vsb", bufs=1) as vsb,
            tc.tile_pool(name="ep", bufs=12) as ep,
            tc.tile_pool(name="rsp", bufs=4) as rsp,
            tc.tile_pool(name="ynp", bufs=1) as ynp,
            tc.tile_pool(name="ytp", bufs=1) as ytp,
            tc.tile_pool(name="outp", bufs=16) as outp,
            tc.tile_pool(name="psA", bufs=2, space="PSUM") as psA,
            tc.tile_pool(name="psE", bufs=2, space="PSUM") as psE,
            tc.tile_pool(name="psY", bufs=2, space="PSUM") as psY,
        ):
            # ---- first DMAs on the critical path: wqk + x quarter 0,
            # interleaved in 1024-col chunks so the first B matmuls can
            # start after ~2 chunks ----
            xt_sb = [None] * NQ
            xt_sb[0] = xtp.tile([128, 4096], BF16, name="xt0", tag="xt0")
            wqk_sb = wts.tile([128, 4096], BF16, name="wqk_sb")
            # single SP queue, arrival-ordered (the DMA engine pool is a
            # serialized resource, so issue order == arrival order). wqk is
            # p-major on the host, so B(0) p0 only needs its first 1024
            # cols; interleave with xt0 chunks so B p0 finishes ~5us in.
            nc.sync.dma_start(out=wqk_sb[:, :1024], in_=wqk_d[:, :1024])
            for ch in range(4):
                sl = slice(ch * 1024, (ch + 1) * 1024)
                nc.sync.dma_start(out=xt_sb[0][:, sl], in_=xt_d[:128, sl])
            nc.sync.dma_start(out=wqk_sb[:, 2048:3072],
                              in_=wqk_d[:, 2048:3072])  # p2 (k of heads 0,1)
            wv_sb = wts.tile([128, 8 * 260], BF16, name="wv_sb")
            nc.sync.dma_start(out=wv_sb[:], in_=wv_d[:])
            wvl_sb = wts.tile([1, 260], BF16, name="wvl_sb")
            nc.sync.dma_start(out=wvl_sb[:], in_=wvl_d[:])
            nc.sync.dma_start(out=wqk_sb[:, 1024:2048],
                              in_=wqk_d[:, 1024:2048])  # p1
            nc.sync.dma_start(out=wqk_sb[:, 3072:], in_=wqk_d[:, 3072:])  # p3
            bqk_sb = const.tile([128, 4], F32, name="bqk_sb")
            nc.sync.dma_start(out=bqk_sb[:], in_=bqk_d[:])
            for q in range(1, NQ):
                xt_sb[q] = xtp.tile([128, 4096], BF16, name=f"xt{q}", tag=f"xt{q}")
            nc.sync.dma_start(out=xt_sb[1][:], in_=xt_d[128:256, :])
            wp_sb = wts.tile([128, 2048], BF16, name="wp_sb")
            nc.sync.dma_start(out=wp_sb[:], in_=wp_d[:])
            nc.sync.dma_start(out=xt_sb[2][:], in_=xt_d[256:384, :])
            nc.sync.dma_start(out=xt_sb[3][:], in_=xt_d[384:512, :])

            # ---- constants ----
            # gpsimd can't write bf16; build f32 then DVE copy-cast.
            ident32 = const.tile([128, 128], F32, name="ident32")
            make_identity(nc, ident32)
            ident = const.tile([128, 128], BF16, name="ident")
            nc.vector.tensor_copy(ident[:], ident32[:])
            # maskT[a, b] = 0 where a >= b else -8000; used as lhsT so the
            # psum receives M[j, i] = maskT[i, j] = 0 iff i >= j.
            maskf32 = const.tile([128, 128], F32, name="maskf32")
            nc.gpsimd.memset(maskf32[:], 0.0)
            nc.gpsimd.affine_select(
                out=maskf32[:], in_=maskf32[:],
                compare_op=mybir.AluOpType.is_ge, fill=-8000.0,
                base=0, channel_multiplier=1, pattern=[[-1, 128]],
            )
            maskT = const.tile([128, 128], BF16, name="maskT")
            nc.vector.tensor_copy(maskT[:], maskf32[:])
            ones32 = const.tile([1, 128], F32, name="ones32")
            nc.gpsimd.memset(ones32[:], 1.0)
            ones_b = const.tile([1, 128], BF16, name="ones_b")
            nc.vector.tensor_copy(ones_b[:], ones32[:])

            # ---- persistent activations ----
            qkT = [qkt.tile([128, T], BF16, name=f"qkT{p}", tag=f"qkT{p}")
                   for p in range(4)]
            v_sb = [vsb.tile([128, 260], BF16, name=f"v{jt}", tag=f"v{jt}")
                    for jt in range(NT)]

            # ---------------- emission units ----------------
            # filler units: (est_pe_ns, closure). Emitted between attention
            # stages to keep PE busy while ACT chews exp.
            def b_group(Q, p):
                def emit():
                    ps = psA.tile([128, 512], F32, name="ps_qk", tag="psA")
                    for kt in range(NKT):
                        nc.tensor.matmul(
                            ps[:],
                            wqk_sb[:, p * 1024 + kt * 128: p * 1024 + (kt + 1) * 128],
                            xt_sb[Q][:, kt * 512:(kt + 1) * 512],
                            start=(kt == 0), stop=(kt == NKT - 1),
                        )
                    nc.vector.tensor_scalar_add(
                        qkT[p][:, Q * 512:(Q + 1) * 512], ps[:],
                        bqk_sb[:, p:p + 1])
                return (8 * MM_NS, emit)

            def c_group(Q, mtl):
                def emit():
                    jt = 4 * Q + mtl
                    ps = psA.tile([128, 260], F32, name="ps_v", tag="psA")
                    for kt in range(NKT):
                        nc.tensor.matmul(
                            ps[:],
                            xt_sb[Q][:, kt * 512 + mtl * 128: kt * 512 + (mtl + 1) * 128],
                            wv_sb[:, kt * 260:(kt + 1) * 260],
                            start=(kt == 0), stop=False,
                        )
                    nc.tensor.matmul(ps[:], ones_b[:], wvl_sb[:],
                                     start=False, stop=True)
                    nc.vector.tensor_copy(v_sb[jt][:], ps[:])
                return (9 * 260 * 0.4167, emit)

            y_norm = {}   # (ic, itl) -> tile
            yT = {}       # (ic, kt) -> tile

            def tre_group(ic, mtl, tail=False):
                """Transpose y_norm[ic, mtl] into yT and project+store.
                yT[(ic,)] is one [128, 1024] tile: kt block at col kt*512.
                tail=True spreads evictions across DVE and ACT (end of
                kernel, ACT is idle)."""
                def emit():
                    mt = 4 * ic + mtl
                    pst = psA.tile([128, 256], BF16, name="ps_tr", tag="psA")
                    for kt in range(2):
                        nc.tensor.transpose(
                            pst[:, kt * 128:(kt + 1) * 128],
                            y_norm[(ic, mtl)][:, kt * 128:(kt + 1) * 128],
                            ident[:])
                    # one strided evict writes both kt blocks of yT
                    dst = yT[(ic,)][:].rearrange(
                        "p (a b) -> p a b", a=2)[:, :, mtl * 128:(mtl + 1) * 128]
                    if tail:
                        nc.scalar.copy(dst, pst[:])
                    else:
                        nc.vector.tensor_copy(dst, pst[:])
                    o = outp.tile([128, 1024], BF16, name="o_t", tag="o_t")
                    for nch in range(2):
                        ps = psA.tile([128, 512], F32, name="ps_o", tag="psA")
                        for kt in range(2):
                            nc.tensor.matmul(
                                ps[:],
                                yT[(ic,)][:, kt * 512 + mtl * 128:
                                          kt * 512 + (mtl + 1) * 128],
                                wp_sb[:, kt * 1024 + nch * 512: kt * 1024 + (nch + 1) * 512],
                                start=(kt == 0), stop=(kt == 1),
                            )
                        if tail and nch == 0:
                            nc.scalar.copy(o[:, nch * 512:(nch + 1) * 512], ps[:])
                        else:
                            nc.vector.tensor_copy(
                                o[:, nch * 512:(nch + 1) * 512], ps[:])
                        nc.sync.dma_start(
                            out=out_d[mt * 128:(mt + 1) * 128,
                                      nch * 512:(nch + 1) * 512],
                            in_=o[:, nch * 512:(nch + 1) * 512])
                return (2 * 128 * 0.4167 + 4 * MM_NS, emit)

            def itl_key(mtl):
                return mtl

            # global filler deque: (pe_ns, emit, marker). markers order
            # dependencies: ("B", ic) must emit before attn(ic)'s att reads
            # qkT; ("C", ic) before attn(ic)'s diagonal AVs read v.
            filler = []
            consumed = {("B", 0)}
            # global emission clock (ns estimates): pe = PE busy frontier,
            # act = ACT (exp) completion frontier. Used to decide when PE
            # needs filler so it never idles waiting for exp.
            clk = {"pe": 0.0, "act": 0.0}
            SEM_LAT = 100.0
            tail_mode = [False]

            def pop_one():
                pe_ns, emit, marker, ready = filler.pop(0)
                emit()
                if marker:
                    consumed.add(marker)
                clk["pe"] += pe_ns

            def pop_filler(need_pe_ns):
                got = 0.0
                while filler and got < need_pe_ns:
                    if filler[0][3] > clk["pe"]:
                        break  # head unit's inputs not DMA'd yet
                    pe_ns = filler[0][0]
                    pop_one()
                    got += pe_ns
                return got

            def advance_pe_to(t):
                """PE must reach estimated time t before the next emitted
                instruction can run: emit filler to cover the wait."""
                while clk["pe"] < t and filler and filler[0][3] <= clk["pe"]:
                    pop_one()
                if clk["pe"] < t:
                    clk["pe"] = t  # PE idles

            def force_pop(marker):
                while marker not in consumed and filler:
                    pop_one()

            # ---------------- attention head ----------------
            LAG = 3

            def atthead(s, ic, prev_fin=None, prev_fin_ready=0.0):
                if ic == 0 and s == 2:
                    # heads 2,3 read qkT p1/p3, which ride the filler
                    force_pop(("B0hi", 0))
                qrow = (s % 2) * 64
                qtile = qkT[s // 2]
                ktile = qkT[2 + s // 2]
                ps_y = psY.tile([128, 260], F32, name="ps_y", tag="psY")
                n_av = [0] * 4            # AVs emitted per i-tile region
                tot_av = [4 * ic + itl + 1 for itl in range(4)]
                # jt -> (e tile, col of i-block 0); filled as stages emit
                e_of = {}

                def av(itl, jt):
                    # PSUM groups are bank(2KB)-granular: regions of ps_y
                    # accumulate strictly one group at a time (r0 rolls with
                    # the stages; r1..r3 burst after all e tiles exist).
                    if jt >= 4 * ic:
                        force_pop(("C", ic))
                    e, base = e_of[jt]
                    nc.tensor.matmul(
                        ps_y[:, itl * 65:itl * 65 + 65],
                        e[:, base + itl * 128: base + itl * 128 + 128],
                        v_sb[jt][:, s * 65:s * 65 + 65],
                        start=(n_av[itl] == 0),
                        stop=(n_av[itl] == tot_av[itl] - 1),
                    )
                    n_av[itl] += 1

                # stage list: full pairs first, then the diagonal halves.
                att_stages = []

                for pr in range(2 * ic):
                    def mk_att(pr=pr):
                        ps_a = psE.tile([128, 1024], F32, name="ps_a", tag="psE")
                        e = ep.tile([128, 1024], BF16, name="e_t", tag="e_t")
                        for h in range(2):
                            jt = 2 * pr + h
                            nc.tensor.matmul(
                                ps_a[:, h * 512:(h + 1) * 512],
                                ktile[qrow:qrow + 64, jt * 128:(jt + 1) * 128],
                                qtile[qrow:qrow + 64, ic * 512:(ic + 1) * 512],
                                start=True, stop=True,
                            )
                            e_of[jt] = (e, h * 512)
                        nc.scalar.activation(
                            e[:], ps_a[:], mybir.ActivationFunctionType.Exp,
                            scale=float(HD) ** -0.5)
                    # stage jts, ACT ns, PE ns
                    att_stages.append((mk_att, [2 * pr, 2 * pr + 1],
                                       EXP_PAIR_NS, 2 * MM_NS))

                for half in range(2):
                    def mk_att(half=half):
                        ps_a = psE.tile([128, 1024], F32, name="ps_a", tag="psE")
                        e = ep.tile([128, 1024], BF16, name="e_t", tag="e_t")
                        for h in range(2):
                            o = (2 * half + h) * 128
                            jt = 4 * ic + 2 * half + h
                            # diagonal 128-col block: att + mask accumulated
                            nc.tensor.matmul(
                                ps_a[:, h * 512 + o: h * 512 + o + 128],
                                ktile[qrow:qrow + 64, jt * 128:(jt + 1) * 128],
                                qtile[qrow:qrow + 64,
                                      ic * 512 + o: ic * 512 + o + 128],
                                start=True, stop=False,
                            )
                            nc.tensor.matmul(
                                ps_a[:, h * 512 + o: h * 512 + o + 128],
                                maskT[:], ident[:],
                                start=False, stop=True,
                            )
                            # unmasked remainder of the row band
                            if o + 128 < 512:
                                nc.tensor.matmul(
                                    ps_a[:, h * 512 + o + 128:(h + 1) * 512],
                                    ktile[qrow:qrow + 64, jt * 128:(jt + 1) * 128],
                                    qtile[qrow:qrow + 64,
                                          ic * 512 + o + 128:(ic + 1) * 512],
                                    start=True, stop=True,
                                )
                            e_of[jt] = (e, h * 512)
                            nc.scalar.activation(
                                e[:, h * 512 + o:(h + 1) * 512],
                                ps_a[:, h * 512 + o:(h + 1) * 512],
                                mybir.ActivationFunctionType.Exp,
                                scale=float(HD) ** -0.5)
                    jts = [4 * ic + 2 * half, 4 * ic + 2 * half + 1]
                    att_pe = (896 - 512 * half) * 0.4167 + 2 * 128 * 0.4167
                    exp_ns = (896 - 512 * half) * 0.833 + 2 * EXP_INIT_NS
                    att_stages.append((mk_att, jts, exp_ns, att_pe))

                # emit: stages; region 0's group rolls along (lagged);
                # regions 1..3 burst after the last stage. The global clock
                # models the psE ring (bufs=2): att stage k reuses stage
                # k-2's psum, so PE must not reach att(k) before exp(k-2)
                # completes -- filler covers the difference.
                AV_NS = 65 * 0.4167
                done = []       # per stage: jts whose e exists
                exp_done = []   # per stage: est. exp completion time
                fin_emitted = [prev_fin is None]
                for k, (mk, jts, exp_ns, att_pe) in enumerate(att_stages):
                    sid = len(stage_registry)
                    stage_registry.append((sid, nc.next_id()))
                    h = hints.get(sid, 0.0)
                    if h > 0:
                        pop_filler(h)
                    if k >= 2:
                        advance_pe_to(exp_done[k - 2])
                    mk()
                    clk["pe"] += att_pe
                    clk["act"] = max(clk["act"],
                                     clk["pe"] + SEM_LAT) + exp_ns
                    exp_done.append(clk["act"] + SEM_LAT)
                    done.append(jts)
                    if k >= LAG:
                        for jt in done[k - LAG]:
                            if jt <= 4 * ic:
                                av(0, jt)
                                clk["pe"] += AV_NS
                    # previous head's deferred tail: emit once its exps are
                    # surely done (covered by our att stream, not filler)
                    if not fin_emitted[0] and (
                            clk["pe"] >= prev_fin_ready
                            or k >= len(att_stages) - 2):
                        prev_fin()
                        fin_emitted[0] = True
                if not fin_emitted[0]:
                    advance_pe_to(prev_fin_ready)
                    prev_fin()

                def fin():
                    # tail: r0 leftovers + r1 only need the half0 diag exp;
                    # r2/r3 additionally need half1 -- gate separately so
                    # the early regions stream while half1's exp finishes.
                    if len(exp_done) >= 2:
                        advance_pe_to(exp_done[-2])
                    for jts in done[max(0, len(done) - LAG):]:
                        for jt in jts:
                            if jt <= 4 * ic:
                                av(0, jt)
                                clk["pe"] += AV_NS
                    for jt in range(4 * ic + 2):
                        av(1, jt)
                        clk["pe"] += AV_NS
                    if exp_done:
                        advance_pe_to(exp_done[-1])
                    for itl in range(2, 4):
                        for jt in range(4 * ic + itl + 1):
                            av(itl, jt)
                            clk["pe"] += AV_NS

                    # normalize: rs = 1/S per i-tile, per-partition scalars
                    rs = rsp.tile([128, 4], F32, name="rs_t", tag="rs_t")
                    with nc.allow_low_precision(
                            reason="softmax sum reciprocal"):
                        nc.vector.reciprocal(rs[:], ps_y[:, 64::65])
                    for itl in range(4):
                        yn = y_norm.get((ic, itl))
                        if yn is None:
                            yn = ynp.tile([128, 256], BF16, name="y_n",
                                          tag=f"yn{(ic % 2) * 4 + itl}")
                            y_norm[(ic, itl)] = yn
                        if tail_mode[0] and itl % 2 == 1:
                            # end of kernel: ACT is idle, split the norms
                            nc.scalar.mul(
                                yn[:, s * 64:(s + 1) * 64],
                                ps_y[:, itl * 65:itl * 65 + 64],
                                rs[:, itl:itl + 1])
                        else:
                            nc.vector.tensor_scalar(
                                out=yn[:, s * 64:(s + 1) * 64],
                                in0=ps_y[:, itl * 65:itl * 65 + 64],
                                scalar1=rs[:, itl:itl + 1],
                                scalar2=None,
                                op0=mybir.AluOpType.mult,
                            )
                return fin, (exp_done[-2] if len(exp_done) >= 2 else 0.0)

            # ---------------- schedule ----------------
            def attn_quarter(ic):
                force_pop(("B", ic))
                yT[(ic,)] = ytp.tile([128, 1024], BF16, name="yT_t",
                                     tag=f"yT{ic % 2}")
                fin, ready = None, 0.0
                for s in range(4):
                    fin, ready = atthead(s, ic, fin, ready)
                    if ic == 3 and s == 2:
                        # fin(s=1) was just emitted inside atthead(2).
                        # yT's kt=0 half only reads heads 0,1 (cols 0..127
                        # of y_norm): transpose it now so it fills the last
                        # head's exp window and shortens the final tail.
                        for m in range(4):
                            pst = psA.tile([128, 128], BF16,
                                           name="ps_tr", tag="psA")
                            nc.tensor.transpose(
                                pst[:], y_norm[(3, m)][:, 0:128], ident[:])
                            nc.vector.tensor_copy(
                                yT[(3,)][:, m * 128:(m + 1) * 128], pst[:])
                        clk["pe"] += 4 * 128 * 0.4167
                # last head's tail (fin gates its own exp waits with filler)
                if ic == 3:
                    tail_mode[0] = True
                fin()

            # it0: B(0) p0/p2 staged by kt-halves so PE starts on the first
            # DMA arrivals. Heads 0,1 of attn(0) only need p0 (their q) and
            # p2 (their k), so attention -- and with it ACT's exp stream --
            # can start ~5us earlier; B(0) p1/p3 and C(0) ride the filler.
            psB0 = []
            for p in (0, 2):
                ps = psA.tile([128, 512], F32, name="ps_qk", tag="psA")
                for kt in range(4):
                    nc.tensor.matmul(
                        ps[:],
                        wqk_sb[:, p * 1024 + kt * 128: p * 1024 + (kt + 1) * 128],
                        xt_sb[0][:, kt * 512:(kt + 1) * 512],
                        start=(kt == 0), stop=False,
                    )
                psB0.append((p, ps))
            for p, ps in psB0:
                for kt in range(4, NKT):
                    nc.tensor.matmul(
                        ps[:],
                        wqk_sb[:, p * 1024 + kt * 128: p * 1024 + (kt + 1) * 128],
                        xt_sb[0][:, kt * 512:(kt + 1) * 512],
                        start=False, stop=(kt == NKT - 1),
                    )
                nc.vector.tensor_scalar_add(
                    qkT[p][:, :512], ps[:], bqk_sb[:, p:p + 1])
            # startup estimate: DMA lead-in + B(0) p0/p2 at low pstate
            clk["pe"] = 8000.0
            filler.append(b_group(0, 1) + (None, 0.0))
            filler.append(b_group(0, 3) + (("B0hi", 0), 0.0))
            for mtl in range(4):
                filler.append(c_group(0, mtl) + (
                    (("C", 0),) if mtl == 3 else (None,)) + (0.0,))

            # everything else rides the filler queue, FIFO-ordered so
            # earlier-needed work is popped first.
            # ready = rough DMA arrival estimate of xt[Q] (ns)
            XT_READY = {1: 12000.0, 2: 16000.0, 3: 19000.0}
            for Q in range(1, NQ):
                rdy = XT_READY[Q]
                filler.append(b_group(Q, 0) + (None, rdy))
                filler.append(b_group(Q, 1) + (None, rdy))
                filler.append(b_group(Q, 2) + (None, rdy))
                filler.append(b_group(Q, 3) + (("B", Q), rdy))
                filler.append(c_group(Q, 0) + (None, rdy))
                filler.append(c_group(Q, 1) + (None, rdy))
                filler.append(c_group(Q, 2) + (None, rdy))
                filler.append(c_group(Q, 3) + (("C", Q), rdy))

            attn_quarter(0)
            filler += [tre_group(0, m) + (None, clk["pe"] + 1500.0) for m in range(4)]
            attn_quarter(1)
            filler += [tre_group(1, m) + (None, clk["pe"] + 1500.0) for m in range(4)]
            attn_quarter(2)
            filler += [tre_group(2, m) + (None, clk["pe"] + 16000.0) for m in range(4)]
            attn_quarter(3)
            pop_filler(1e9)
            # batched final TRE: the kt=1 transposes+evicts first (their
            # norm waits pipeline on DVE/ACT), then all projections+stores.
            for m in range(4):
                pst = psA.tile([128, 128], BF16, name="ps_tr", tag="psA")
                nc.tensor.transpose(
                    pst[:], y_norm[(3, m)][:, 128:256], ident[:])
                dst = yT[(3,)][:, 512 + m * 128: 512 + (m + 1) * 128]
                if m % 2 == 0:
                    nc.scalar.copy(dst, pst[:])
                else:
                    nc.vector.tensor_copy(dst, pst[:])
            for m in range(4):
                mt = 12 + m
                o = outp.tile([128, 1024], BF16, name="o_t", tag="o_t")
                # exp is done -- reuse the (2-bank) psE tiles so the last
                # four projections pipeline instead of serializing on psA
                ps = psE.tile([128, 1024], F32, name="ps_a", tag="psE")
                for nch in range(2):
                    for kt in range(2):
                        nc.tensor.matmul(
                            ps[:, nch * 512:(nch + 1) * 512],
                            yT[(3,)][:, kt * 512 + m * 128:
                                     kt * 512 + (m + 1) * 128],
                            wp_sb[:, kt * 1024 + nch * 512:
                                  kt * 1024 + (nch + 1) * 512],
                            start=(kt == 0), stop=(kt == 1),
                        )
                    if nch == 0:
                        nc.scalar.copy(o[:, nch * 512:(nch + 1) * 512],
                                       ps[:, nch * 512:(nch + 1) * 512])
                    else:
                        nc.vector.tensor_copy(
                            o[:, nch * 512:(nch + 1) * 512],
                            ps[:, nch * 512:(nch + 1) * 512])
                    nc.sync.dma_start(
                        out=out_d[mt * 128:(mt + 1) * 128,
                                  nch * 512:(nch + 1) * 512],
                        in_=o[:, nch * 512:(nch + 1) * 512])

    if split_waits:
        _split_matmul_waits(nc)
    nc._stage_registry = stage_registry
    return nc


def _split_matmul_waits(nc):
    """Walrus codegen in this pipeline allows only one sync wait per
    instruction for most ISA structs (S3_LW, PSEUDO_DMA_DIRECT2D, S3D3_TS,
    ...). Move extra waits onto inserted NoOps on the same engine (program
    order preserves semantics)."""
    n_split = 0
    for bb in nc.main_func.blocks:
        out = []
        for ins in bb.instructions:
            si = getattr(ins, "sync_info", None)
            if (si is not None and len(si.on_wait) >= 2
                    and type(ins).__name__ != "InstNoOp"):
                for w in si.on_wait[:-1]:
                    nop = mybir.InstNoOp(name=f"I-wsplit-{nc.next_id()}",
                                         ins=[], outs=[])
                    nop.engine = ins.engine
                    nop.sync_info = mybir.SyncInfo(on_wait=[w], on_update=[])
                    out.append(nop)
                    n_split += 1
                ins.sync_info = mybir.SyncInfo(
                    on_wait=[si.on_wait[-1]], on_update=si.on_update)
            out.append(ins)
        bb.instructions[:] = out
    return n_split


def _bf16(a):
    import ml_dtypes
    return np.ascontiguousarray(a.astype(ml_dtypes.bfloat16))


def shard_inputs(x, Wqkv, bqkv, Wproj, bproj):
    x = np.asarray(x, np.float32)
    Wqkv = np.asarray(Wqkv, np.float32)
    bqkv = np.asarray(bqkv, np.float32)
    Wproj = np.asarray(Wproj, np.float32)
    in_maps = []
    xt_b = []
    for b in range(B):
        # xt[Q*128+p, kt*512+m] = x[b][Q*512+m, kt*128+p]
        xT = x[b].T  # [C, T]
        xt = xT.reshape(NKT, 128, NQ, 512).transpose(2, 1, 0, 3).reshape(
            NQ * 128, 4096)
        xt_b.append(_bf16(xt))
    for c in range(N_CORES):
        b, hg = c // 4, c % 4
        wqk = np.concatenate(
            [Wqkv[:, hg * 256:(hg + 1) * 256],
             Wqkv[:, C + hg * 256: C + (hg + 1) * 256]], axis=1)  # [C, 512]
        # p-major: wqk2[p, pc*1024 + kt*128 + m] = wqk[kt*128+p, pc*128+m]
        wqk2 = wqk.reshape(NKT, 128, 4, 128).transpose(1, 2, 0, 3).reshape(
            128, 4096)
        bqk = np.concatenate(
            [bqkv[hg * 256:(hg + 1) * 256],
             bqkv[C + hg * 256: C + (hg + 1) * 256]])  # [512]
        bqk2 = np.ascontiguousarray(bqk.reshape(4, 128).T)  # [128, 4]
        wv = np.zeros((C, 260), np.float32)
        wvl = np.zeros((1, 260), np.float32)
        for s in range(4):
            h = 4 * hg + s
            wv[:, s * 65:s * 65 + 64] = Wqkv[:, 2 * C + h * 64: 2 * C + (h + 1) * 64]
            wvl[0, s * 65:s * 65 + 64] = bqkv[2 * C + h * 64: 2 * C + (h + 1) * 64]
            wvl[0, s * 65 + 64] = 1.0
        wv2 = wv.reshape(NKT, 128, 260).transpose(1, 0, 2).reshape(128, 8 * 260)
        wp = Wproj[hg * 256:(hg + 1) * 256, :]  # [256, C]
        wp2 = wp.reshape(2, 128, C).transpose(1, 0, 2).reshape(128, 2048)
        in_maps.append({
            "xt": xt_b[b],
            "wqk": _bf16(wqk2),
            "bqk": np.ascontiguousarray(bqk2, dtype=np.float32),
            "wv": _bf16(wv2),
            "wvl": _bf16(wvl),
            "wp": _bf16(wp2),
        })
    return in_maps


_NC_CACHE = {}


def kernel(x, Wqkv, bqkv, Wproj, bproj):
    from concourse.bass_utils import run_bass_kernel_spmd

    if "nc" not in _NC_CACHE:
        _NC_CACHE["nc"] = build_nc()
    nc = _NC_CACHE["nc"]
    in_maps = shard_inputs(x, Wqkv, bqkv, Wproj, bproj)
    res = run_bass_kernel_spmd(nc, in_maps, list(range(N_CORES)))
    _NC_CACHE["last_exec_time_ns"] = res.exec_time_ns
    bproj = np.asarray(bproj, np.float32)
    out = np.zeros((B, T, C), np.float32)
    for c in range(N_CORES):
        out[c // 4] += np.asarray(res.results[c]["out"], np.float32)
    out += bproj[None, None, :]
    return out



# revision 27
# speedup vs baseline: 1.0554x; 1.0554x over previous
"""Causal self-attention (B=2, T=2048, C=1024, NH=16, HD=64) on 8 NeuronCores.

Sharding: core c -> (batch b = c//4, head-group hg = c%4 of 4 heads).
Each core computes the qkv projection for its 4 heads from x[b], attention
for its 4 (b,h) units, and a partial output projection (row-parallel over
the head dim). Unshard = sum of the 4 partials per batch + bproj (host).

v2 design (all matmul operands bf16, PSUM accumulation f32):
  - Host pre-transposes x and pre-swizzles every weight into the exact
    SBUF layout, so DMAs are plain [128, W] copies and the device does no
    layout work at all (the old on-PE x-transpose stage is gone).
  - B: qkT [512, T] = wqk.T @ xT + bqk, evicted per quarter as bf16.
  - C: v_aug [T, 260] = [x | 1] @ wv_aug; per head 64 v columns + a ones
    column so the softmax row-sums fall out of the AV matmul for free.
  - D: att^T[j,i] per (head, i-chunk of 512) on PE (K=64); causal mask is
    a constant [128,128] lower-triangle(-8000) tile ACCUMULATED into the
    diagonal att psum before exp (no vector-engine masking). exp on ACT
    (scale 1/8, no max subtraction -- logits are O(1) by construction),
    output bf16.
  - AV flipped: y[i-tile, 4*65] accumulated as e_block.T @ v (e is the
    stationary operand), so each j-tile costs 65 moving rows instead of
    128+. Row 64 of each head's 65-col group = softmax sum S (from the
    ones column). Normalize with per-partition scalars (reciprocal + DVE
    tensor_scalar), transpose y on PE, project: out = yT.T @ wp (bf16
    partial, no bias; host adds bproj once).
  - The emission order software-pipelines ACT(exp) against PE: attention
    stages for quarter ic interleave with B/C work of quarter ic+1 and
    the transpose+projection of quarter ic-1 via a filler queue.
"""
import os
import sys

import numpy as np

for _p in ("/opt/trn_rl_repo",):
    if _p not in sys.path and os.path.isdir(_p):
        sys.path.insert(0, _p)

import concourse.bass as bass
import concourse.mybir as mybir
import concourse.tile as tile
from concourse.masks import make_identity

B, T, C, NH, HD = 2, 2048, 1024, 16, 64
F32 = mybir.dt.float32
BF16 = mybir.dt.bfloat16
FP8 = mybir.dt.float8e4
N_CORES = 8
NQ = 4          # token quarters (512 tokens each)
NKT = C // 128  # 8 contraction tiles
NT = T // 128   # 16 token tiles

# w (qkv) host-prescale: lifts fp8 hi/lo residuals of the N(0, 1/C)
# weights above the e4m3 subnormal floor. q,k,v come out 32x larger;
# the exp scale absorbs 32*32 for qk, and the v ones-column (also
# scaled) makes the softmax division self-normalizing.
WSCALE = 32.0
MASK_VAL = -8000.0 * WSCALE * WSCALE

# cost-model estimates (ns) used only to balance the filler interleave
MM_NS = 512 * 0.4167          # 512-row bf16 matmul
DRB_NS = 12 * 512 * 0.5 * 0.4167   # one B group: 12 DoubleRow matmuls
DRC_NS = (12 * 260 * 0.5 + 260) * 0.4167  # one C group incl bf16 bias mm
EXP_INIT_NS = 160.0           # per-exp-instruction access overhead
EXP_PAIR_NS = 1024 * 0.833 + EXP_INIT_NS
N_WARMUP = 9                  # f32 128-col warmup matmuls (PE clock ramp)


# measured-feedback filler pops (stage_seq_id -> ns), from iterating
# TimelineSim: simulate, map PE stalls to emission points, re-pop there.
_STAGE_HINTS = {}


def build_nc(split_waits=True, hints=None, n_warmup=None,
             tre_ready=(1500.0, 20000.0, 26000.0), xt_ready=None):
    # hints: {stage_seq_id: extra_filler_ns} -- measured-feedback pops
    hints = _STAGE_HINTS if hints is None else hints
    N_WARMUP = n_warmup if n_warmup is not None else globals()["N_WARMUP"]
    stage_registry = []   # (stage_seq_id, first_inst_num) in emission order
    nc = bass.Bass()
    # fp8 hi/lo pair layouts (half-major so hi planes can DMA first; wqk
    # stores p in order (0,2,1,3) so the heads-0/1 q+k planes are the
    # leading 2KB of each half -- one DMA each):
    #   xt : [NQ*128, half*4096 + kt*512 + m]          half 0=hi 1=lo
    #   wqk: [128, half*4096 + pc*1024 + kt*128 + m]   half 0=lo 1=hi
    #   wv : [128, half*2080 + kt*260 + n]             half 0=lo 1=hi
    xt_d = nc.declare_dram_parameter("xt", [NQ * 128, 8192], FP8, isOutput=False)
    wqk_d = nc.declare_dram_parameter("wqk", [128, 8192], FP8, isOutput=False)
    bqk_d = nc.declare_dram_parameter("bqk", [128, 4], F32, isOutput=False)
    wv_d = nc.declare_dram_parameter("wv", [128, 2 * 8 * 260], FP8, isOutput=False)
    wvl_d = nc.declare_dram_parameter("wvl", [1, 260], BF16, isOutput=False)
    wp_d = nc.declare_dram_parameter("wp", [128, 2048], BF16, isOutput=False)
    out_d = nc.declare_dram_parameter("out", [T, C], BF16, isOutput=True)
    DR = mybir.MatmulPerfMode.DoubleRow

    with tile.TileContext(nc) as tc:
        with (
            tc.tile_pool(name="const", bufs=1) as const,
            tc.tile_pool(name="wts", bufs=1) as wts,
            tc.tile_pool(name="xtp", bufs=1) as xtp,
            tc.tile_pool(name="qkt", bufs=1) as qkt,
            tc.tile_pool(name="vsb", bufs=1) as vsb,
            tc.tile_pool(name="ep", bufs=12) as ep,
            tc.tile_pool(name="rsp", bufs=4) as rsp,
            tc.tile_pool(name="ynp", bufs=1) as ynp,
            tc.tile_pool(name="ytp", bufs=1) as ytp,
            tc.tile_pool(name="outp", bufs=16) as outp,
            tc.tile_pool(name="psA", bufs=2, space="PSUM") as psA,
            tc.tile_pool(name="psE", bufs=2, space="PSUM") as psE,
            tc.tile_pool(name="psY", bufs=2, space="PSUM") as psY,
        ):
            # ---- first DMAs on the critical path: wqk + x quarter 0.
            # Half-major fp8 layout: hi planes stream first so B(0)'s main
            # (hi*hi) matmuls can start ~1.2us in; lo planes follow and the
            # cross terms accumulate into the same psum group before stop.
            # warmup operand: DVE memset so the PE can start ~0.4us in
            wu = const.tile([128, 128], F32, name="wu")
            nc.vector.memset(wu[:], 0.25)
            xt_sb = [None] * NQ
            xt_sb[0] = xtp.tile([128, 8192], FP8, name="xt0", tag="xt0")
            wqk_sb = wts.tile([128, 8192], FP8, name="wqk_sb")
            nc.sync.dma_start(out=wqk_sb[:, 4096:6144],
                              in_=wqk_d[:, 4096:6144])  # p0+p2 hi (heads 0,1)
            nc.sync.dma_start(out=xt_sb[0][:, :2048], in_=xt_d[:128, :2048])
            nc.sync.dma_start(out=xt_sb[0][:, 2048:4096],
                              in_=xt_d[:128, 2048:4096])  # xt0 hi
            nc.sync.dma_start(out=wqk_sb[:, :2048],
                              in_=wqk_d[:, :2048])  # p0+p2 lo
            nc.sync.dma_start(out=xt_sb[0][:, 4096:6144],
                              in_=xt_d[:128, 4096:6144])
            nc.sync.dma_start(out=xt_sb[0][:, 6144:8192],
                              in_=xt_d[:128, 6144:8192])  # xt0 lo
            wv_sb = wts.tile([128, 2 * 8 * 260], FP8, name="wv_sb")
            nc.sync.dma_start(out=wv_sb[:], in_=wv_d[:])
            wvl_sb = wts.tile([1, 260], BF16, name="wvl_sb")
            nc.sync.dma_start(out=wvl_sb[:], in_=wvl_d[:])
            nc.sync.dma_start(out=wqk_sb[:, 6144:8192],
                              in_=wqk_d[:, 6144:8192])  # p1+p3 hi
            nc.sync.dma_start(out=wqk_sb[:, 2048:4096],
                              in_=wqk_d[:, 2048:4096])  # p1+p3 lo
            bqk_sb = const.tile([128, 4], F32, name="bqk_sb")
            nc.sync.dma_start(out=bqk_sb[:], in_=bqk_d[:])
            for q in range(1, NQ):
                xt_sb[q] = xtp.tile([128, 8192], FP8, name=f"xt{q}", tag=f"xt{q}")
            nc.sync.dma_start(out=xt_sb[1][:], in_=xt_d[128:256, :])
            wp_sb = wts.tile([128, 2048], BF16, name="wp_sb")
            nc.sync.dma_start(out=wp_sb[:], in_=wp_d[:])
            nc.sync.dma_start(out=xt_sb[2][:], in_=xt_d[256:384, :])
            nc.sync.dma_start(out=xt_sb[3][:], in_=xt_d[384:512, :])

            # ---- PE warmup: the cost model ramps the PE to full clock only
            # after ~3us of continuous busy. Burn f32 matmuls on a memset
            # tile during the DMA lead-in so real work starts at full speed.
            psW = psA.tile([128, 128], F32, name="ps_w", tag="psA")
            for r in range(N_WARMUP):
                nc.tensor.matmul(psW[:], wu[:], wu[:],
                                 start=(r == 0), stop=(r == N_WARMUP - 1))

            # ---- constants ----
            # gpsimd can't write bf16; build f32 then DVE copy-cast.
            ident32 = const.tile([128, 128], F32, name="ident32")
            make_identity(nc, ident32)
            ident = const.tile([128, 128], BF16, name="ident")
            nc.vector.tensor_copy(ident[:], ident32[:])
            # maskT[a, b] = 0 where a >= b else MASK_VAL; used as lhsT so the
            # psum receives M[j, i] = maskT[i, j] = 0 iff i >= j.
            maskf32 = const.tile([128, 128], F32, name="maskf32")
            nc.gpsimd.memset(maskf32[:], 0.0)
            nc.gpsimd.affine_select(
                out=maskf32[:], in_=maskf32[:],
                compare_op=mybir.AluOpType.is_ge, fill=MASK_VAL,
                base=0, channel_multiplier=1, pattern=[[-1, 128]],
            )
            maskT = const.tile([128, 128], BF16, name="maskT")
            nc.vector.tensor_copy(maskT[:], maskf32[:])
            ones32 = const.tile([1, 128], F32, name="ones32")
            nc.gpsimd.memset(ones32[:], 1.0)
            ones_b = const.tile([1, 128], BF16, name="ones_b")
            nc.vector.tensor_copy(ones_b[:], ones32[:])

            # ---- fp8 pair access-pattern helpers ----
            PCOL = {0: 0, 2: 1, 1: 2, 3: 3}   # p -> column block in wqk

            def wqk_main(p, kp):
                # (w_hi[2kp], w_hi[2kp+1])  [128, 2, 128]
                off = 4096 + PCOL[p] * 1024 + (2 * kp) * 128
                return wqk_sb[:, off:off + 256].rearrange(
                    "a (two m) -> a two m", two=2)

            def wqk_cross(p, kt):
                # (w_lo[kt], w_hi[kt])  dim1 stride 4096; pairs with
                # xt_cross's (x_hi, x_lo) -> lo*hi + hi*lo
                return wqk_sb[:].rearrange(
                    "a (h pkm) -> a h pkm", h=2)[
                        :, :, PCOL[p] * 1024 + kt * 128:
                        PCOL[p] * 1024 + (kt + 1) * 128]

            def xt_main(Q, kp, lo=0, n=512):
                # (x_hi[2kp], x_hi[2kp+1])  [128, 2, n]
                return xt_sb[Q][:].rearrange(
                    "a (h kt m) -> a h kt m", h=2, kt=8)[
                        :, 0, 2 * kp:2 * kp + 2, lo:lo + n]

            def xt_cross(Q, kt, lo=0, n=512):
                # (x_hi[kt], x_lo[kt])  dim1 stride 4096
                return xt_sb[Q][:].rearrange(
                    "a (h km) -> a h km", h=2)[
                        :, :, kt * 512 + lo:kt * 512 + lo + n]

            def wv_main(kp):
                off = 2080 + (2 * kp) * 260
                return wv_sb[:, off:off + 520].rearrange(
                    "a (two m) -> a two m", two=2)

            def wv_cross(kt):
                # (wv_lo[kt], wv_hi[kt])  dim1 stride 2080
                return wv_sb[:].rearrange(
                    "a (h kn) -> a h kn", h=2)[:, :, kt * 260:(kt + 1) * 260]

            # ---- persistent activations ----
            qkT = [qkt.tile([128, T], BF16, name=f"qkT{p}", tag=f"qkT{p}")
                   for p in range(4)]
            v_sb = [vsb.tile([128, 260], BF16, name=f"v{jt}", tag=f"v{jt}")
                    for jt in range(NT)]

            # ---------------- emission units ----------------
            # filler units: (est_pe_ns, closure). Emitted between attention
            # stages to keep PE busy while ACT chews exp.
            def b_group(Q, p):
                # qkT[p, Q] = (w_hi+w_lo).T (x_hi+x_lo) - w_lo.T x_lo over
                # K=1024: 4 DoubleRow mains (kt pairs, hi*hi) + 8 crosses.
                def emit():
                    ps = psA.tile([128, 512], F32, name="ps_qk", tag="psA")
                    for kp in range(4):
                        nc.tensor.matmul(
                            ps[:], wqk_main(p, kp), xt_main(Q, kp),
                            start=(kp == 0), stop=False, perf_mode=DR)
                    for kt in range(NKT):
                        nc.tensor.matmul(
                            ps[:], wqk_cross(p, kt), xt_cross(Q, kt),
                            start=False, stop=(kt == NKT - 1), perf_mode=DR)
                    nc.vector.tensor_scalar_add(
                        qkT[p][:, Q * 512:(Q + 1) * 512], ps[:],
                        bqk_sb[:, p:p + 1])
                return (DRB_NS, emit)

            def c_group(Q, mtl):
                def emit():
                    jt = 4 * Q + mtl
                    ps = psA.tile([128, 260], F32, name="ps_v", tag="psA")
                    for kp in range(4):
                        nc.tensor.matmul(
                            ps[:], xt_main(Q, kp, lo=mtl * 128, n=128),
                            wv_main(kp),
                            start=(kp == 0), stop=False, perf_mode=DR)
                    for kt in range(NKT):
                        nc.tensor.matmul(
                            ps[:], xt_cross(Q, kt, lo=mtl * 128, n=128),
                            wv_cross(kt),
                            start=False, stop=False, perf_mode=DR)
                    nc.tensor.matmul(ps[:], ones_b[:], wvl_sb[:],
                                     start=False, stop=True)
                    nc.vector.tensor_copy(v_sb[jt][:], ps[:])
                return (DRC_NS, emit)

            y_norm = {}   # (ic, itl) -> tile
            yT = {}       # (ic, kt) -> tile

            def tre_group(ic, mtl, tail=False):
                """Transpose y_norm[ic, mtl] into yT and project+store.
                yT[(ic,)] is one [128, 1024] tile: kt block at col kt*512.
                tail=True spreads evictions across DVE and ACT (end of
                kernel, ACT is idle)."""
                def emit():
                    mt = 4 * ic + mtl
                    pst = psA.tile([128, 256], BF16, name="ps_tr", tag="psA")
                    for kt in range(2):
                        nc.tensor.transpose(
                            pst[:, kt * 128:(kt + 1) * 128],
                            y_norm[(ic, mtl)][:, kt * 128:(kt + 1) * 128],
                            ident[:])
                    # one strided evict writes both kt blocks of yT
                    dst = yT[(ic,)][:].rearrange(
                        "p (a b) -> p a b", a=2)[:, :, mtl * 128:(mtl + 1) * 128]
                    if tail:
                        nc.scalar.copy(dst, pst[:])
                    else:
                        nc.vector.tensor_copy(dst, pst[:])
                    o = outp.tile([128, 1024], BF16, name="o_t", tag="o_t")
                    for nch in range(2):
                        ps = psA.tile([128, 512], F32, name="ps_o", tag="psA")
                        for kt in range(2):
                            nc.tensor.matmul(
                                ps[:],
                                yT[(ic,)][:, kt * 512 + mtl * 128:
                                          kt * 512 + (mtl + 1) * 128],
                                wp_sb[:, kt * 1024 + nch * 512: kt * 1024 + (nch + 1) * 512],
                                start=(kt == 0), stop=(kt == 1),
                            )
                        if tail and nch == 0:
                            nc.scalar.copy(o[:, nch * 512:(nch + 1) * 512], ps[:])
                        else:
                            nc.vector.tensor_copy(
                                o[:, nch * 512:(nch + 1) * 512], ps[:])
                    # one store per m-tile: each DMA costs ~650ns of
                    # serialized SP/HWDGE issue regardless of size
                    nc.sync.dma_start(
                        out=out_d[mt * 128:(mt + 1) * 128, :], in_=o[:])
                return (2 * 128 * 0.4167 + 4 * MM_NS, emit)

            def itl_key(mtl):
                return mtl

            # global filler deque: (pe_ns, emit, marker). markers order
            # dependencies: ("B", ic) must emit before attn(ic)'s att reads
            # qkT; ("C", ic) before attn(ic)'s diagonal AVs read v.
            filler = []
            consumed = {("B", 0)}
            # global emission clock (ns estimates): pe = PE busy frontier,
            # act = ACT (exp) completion frontier. Used to decide when PE
            # needs filler so it never idles waiting for exp.
            clk = {"pe": 0.0, "act": 0.0}
            SEM_LAT = 100.0
            tail_mode = [False]

            def pop_one():
                pe_ns, emit, marker, ready = filler.pop(0)
                emit()
                if marker:
                    consumed.add(marker)
                clk["pe"] += pe_ns

            def pop_filler(need_pe_ns):
                got = 0.0
                while filler and got < need_pe_ns:
                    if filler[0][3] > clk["pe"]:
                        break  # head unit's inputs not DMA'd yet
                    pe_ns = filler[0][0]
                    pop_one()
                    got += pe_ns
                return got

            def advance_pe_to(t):
                """PE must reach estimated time t before the next emitted
                instruction can run: emit filler to cover the wait."""
                while clk["pe"] < t and filler and filler[0][3] <= clk["pe"]:
                    pop_one()
                if clk["pe"] < t:
                    clk["pe"] = t  # PE idles

            def force_pop(marker):
                while marker not in consumed and filler:
                    pop_one()

            # ---------------- attention head ----------------
            LAG = 3

            def atthead(s, ic, prev_fin=None, prev_fin_ready=0.0):
                if ic == 0 and s == 2:
                    # heads 2,3 read qkT p1/p3, which ride the filler
                    force_pop(("B0hi", 0))
                qrow = (s % 2) * 64
                qtile = qkT[s // 2]
                ktile = qkT[2 + s // 2]
                ps_y = psY.tile([128, 260], F32, name="ps_y", tag="psY")
                n_av = [0] * 4            # AVs emitted per i-tile region
                tot_av = [4 * ic + itl + 1 for itl in range(4)]
                # jt -> (e tile, col of i-block 0); filled as stages emit
                e_of = {}

                def av(itl, jt):
                    # PSUM groups are bank(2KB)-granular: regions of ps_y
                    # accumulate strictly one group at a time (r0 rolls with
                    # the stages; r1..r3 burst after all e tiles exist).
                    if jt >= 4 * ic:
                        force_pop(("C", ic))
                    e, base = e_of[jt]
                    nc.tensor.matmul(
                        ps_y[:, itl * 65:itl * 65 + 65],
                        e[:, base + itl * 128: base + itl * 128 + 128],
                        v_sb[jt][:, s * 65:s * 65 + 65],
                        start=(n_av[itl] == 0),
                        stop=(n_av[itl] == tot_av[itl] - 1),
                    )
                    n_av[itl] += 1

                # stage list: full pairs first, then the diagonal halves.
                att_stages = []

                for pr in range(2 * ic):
                    def mk_att(pr=pr):
                        ps_a = psE.tile([128, 1024], F32, name="ps_a", tag="psE")
                        e = ep.tile([128, 1024], BF16, name="e_t", tag="e_t")
                        for h in range(2):
                            jt = 2 * pr + h
                            nc.tensor.matmul(
                                ps_a[:, h * 512:(h + 1) * 512],
                                ktile[qrow:qrow + 64, jt * 128:(jt + 1) * 128],
                                qtile[qrow:qrow + 64, ic * 512:(ic + 1) * 512],
                                start=True, stop=True,
                            )
                            e_of[jt] = (e, h * 512)
                        nc.scalar.activation(
                            e[:], ps_a[:], mybir.ActivationFunctionType.Exp,
                            scale=float(HD) ** -0.5 / (WSCALE * WSCALE))
                    # stage jts, ACT ns, PE ns
                    att_stages.append((mk_att, [2 * pr, 2 * pr + 1],
                                       EXP_PAIR_NS, 2 * MM_NS))

                for half in range(2):
                    def mk_att(half=half):
                        ps_a = psE.tile([128, 1024], F32, name="ps_a", tag="psE")
                        e = ep.tile([128, 1024], BF16, name="e_t", tag="e_t")
                        for h in range(2):
                            o = (2 * half + h) * 128
                            jt = 4 * ic + 2 * half + h
                            # diagonal 128-col block: att + mask accumulated
                            nc.tensor.matmul(
                                ps_a[:, h * 512 + o: h * 512 + o + 128],
                                ktile[qrow:qrow + 64, jt * 128:(jt + 1) * 128],
                                qtile[qrow:qrow + 64,
                                      ic * 512 + o: ic * 512 + o + 128],
                                start=True, stop=False,
                            )
                            nc.tensor.matmul(
                                ps_a[:, h * 512 + o: h * 512 + o + 128],
                                maskT[:], ident[:],
                                start=False, stop=True,
                            )
                            # unmasked remainder of the row band
                            if o + 128 < 512:
                                nc.tensor.matmul(
                                    ps_a[:, h * 512 + o + 128:(h + 1) * 512],
                                    ktile[qrow:qrow + 64, jt * 128:(jt + 1) * 128],
                                    qtile[qrow:qrow + 64,
                                          ic * 512 + o + 128:(ic + 1) * 512],
                                    start=True, stop=True,
                                )
                            e_of[jt] = (e, h * 512)
                            nc.scalar.activation(
                                e[:, h * 512 + o:(h + 1) * 512],
                                ps_a[:, h * 512 + o:(h + 1) * 512],
                                mybir.ActivationFunctionType.Exp,
                                scale=float(HD) ** -0.5 / (WSCALE * WSCALE))
                    jts = [4 * ic + 2 * half, 4 * ic + 2 * half + 1]
                    att_pe = (896 - 512 * half) * 0.4167 + 2 * 128 * 0.4167
                    exp_ns = (896 - 512 * half) * 0.833 + 2 * EXP_INIT_NS
                    att_stages.append((mk_att, jts, exp_ns, att_pe))

                # emit: stages; region 0's group rolls along (lagged);
                # regions 1..3 burst after the last stage. The global clock
                # models the psE ring (bufs=2): att stage k reuses stage
                # k-2's psum, so PE must not reach att(k) before exp(k-2)
                # completes -- filler covers the difference.
                AV_NS = 65 * 0.4167
                done = []       # per stage: jts whose e exists
                exp_done = []   # per stage: est. exp completion time
                fin_emitted = [prev_fin is None]
                for k, (mk, jts, exp_ns, att_pe) in enumerate(att_stages):
                    sid = len(stage_registry)
                    stage_registry.append((sid, nc.next_id()))
                    h = hints.get(sid, 0.0)
                    if h > 0:
                        pop_filler(h)
                    if k >= 2:
                        advance_pe_to(exp_done[k - 2])
                    mk()
                    clk["pe"] += att_pe
                    clk["act"] = max(clk["act"],
                                     clk["pe"] + SEM_LAT) + exp_ns
                    exp_done.append(clk["act"] + SEM_LAT)
                    done.append(jts)
                    if k >= LAG:
                        for jt in done[k - LAG]:
                            if jt <= 4 * ic:
                                av(0, jt)
                                clk["pe"] += AV_NS
                    # previous head's deferred tail: emit once its exps are
                    # surely done (covered by our att stream, not filler)
                    if not fin_emitted[0] and (
                            clk["pe"] >= prev_fin_ready
                            or k >= len(att_stages) - 2):
                        prev_fin()
                        fin_emitted[0] = True
                if not fin_emitted[0]:
                    advance_pe_to(prev_fin_ready)
                    prev_fin()

                def fin():
                    # tail: r0 leftovers + r1 only need the half0 diag exp;
                    # r2/r3 additionally need half1 -- gate separately so
                    # the early regions stream while half1's exp finishes.
                    if len(exp_done) >= 2:
                        advance_pe_to(exp_done[-2])
                    for jts in done[max(0, len(done) - LAG):]:
                        for jt in jts:
                            if jt <= 4 * ic:
                                av(0, jt)
                                clk["pe"] += AV_NS
                    for jt in range(4 * ic + 2):
                        av(1, jt)
                        clk["pe"] += AV_NS
                    if exp_done:
                        advance_pe_to(exp_done[-1])
                    for itl in range(2, 4):
                        for jt in range(4 * ic + itl + 1):
                            av(itl, jt)
                            clk["pe"] += AV_NS

                    # normalize: rs = 1/S per i-tile, per-partition scalars
                    rs = rsp.tile([128, 4], F32, name="rs_t", tag="rs_t")
                    with nc.allow_low_precision(
                            reason="softmax sum reciprocal"):
                        nc.vector.reciprocal(rs[:], ps_y[:, 64::65])
                    for itl in range(4):
                        yn = y_norm.get((ic, itl))
                        if yn is None:
                            yn = ynp.tile([128, 256], BF16, name="y_n",
                                          tag=f"yn{(ic % 2) * 4 + itl}")
                            y_norm[(ic, itl)] = yn
                        if tail_mode[0] and itl % 2 == 1:
                            # end of kernel: ACT is idle, split the norms
                            nc.scalar.mul(
                                yn[:, s * 64:(s + 1) * 64],
                                ps_y[:, itl * 65:itl * 65 + 64],
                                rs[:, itl:itl + 1])
                        else:
                            nc.vector.tensor_scalar(
                                out=yn[:, s * 64:(s + 1) * 64],
                                in0=ps_y[:, itl * 65:itl * 65 + 64],
                                scalar1=rs[:, itl:itl + 1],
                                scalar2=None,
                                op0=mybir.AluOpType.mult,
                            )
                return fin, (exp_done[-2] if len(exp_done) >= 2 else 0.0)

            # ---------------- schedule ----------------
            def attn_quarter(ic):
                force_pop(("B", ic))
                yT[(ic,)] = ytp.tile([128, 1024], BF16, name="yT_t",
                                     tag=f"yT{ic % 2}")
                fin, ready = None, 0.0
                for s in range(4):
                    fin, ready = atthead(s, ic, fin, ready)
                    if ic == 3 and s == 2:
                        # fin(s=1) was just emitted inside atthead(2).
                        # yT's kt=0 half only reads heads 0,1 (cols 0..127
                        # of y_norm): transpose it now so it fills the last
                        # head's exp window and shortens the final tail.
                        for m in range(4):
                            pst = psA.tile([128, 128], BF16,
                                           name="ps_tr", tag="psA")
                            nc.tensor.transpose(
                                pst[:], y_norm[(3, m)][:, 0:128], ident[:])
                            nc.vector.tensor_copy(
                                yT[(3,)][:, m * 128:(m + 1) * 128], pst[:])
                        clk["pe"] += 4 * 128 * 0.4167
                # last head's tail (fin gates its own exp waits with filler)
                if ic == 3:
                    tail_mode[0] = True
                fin()

            # it0: B(0) p0/p2 staged by kt-halves so PE starts on the first
            # DMA arrivals. Heads 0,1 of attn(0) only need p0 (their q) and
            # p2 (their k), so attention -- and with it ACT's exp stream --
            # can start ~5us earlier; B(0) p1/p3 and C(0) ride the filler.
            # hi*hi mains first (only need the hi DMA planes), then the
            # cross terms once the lo planes land.
            psB0 = {}
            for p in (0, 2):
                psB0[p] = psA.tile([128, 512], F32, name="ps_qk", tag="psA")
                for kp in range(4):
                    nc.tensor.matmul(psB0[p][:], wqk_main(p, kp),
                                     xt_main(0, kp),
                                     start=(kp == 0), stop=False, perf_mode=DR)
            for p in (0, 2):
                for kt in range(NKT):
                    nc.tensor.matmul(
                        psB0[p][:], wqk_cross(p, kt), xt_cross(0, kt),
                        start=False, stop=(kt == NKT - 1), perf_mode=DR)
                nc.vector.tensor_scalar_add(
                    qkT[p][:, :512], psB0[p][:], bqk_sb[:, p:p + 1])
            # startup estimate: DMA lead-in + warmup + B(0) p0/p2
            clk["pe"] = 6000.0
            filler.append(b_group(0, 1) + (None, 12500.0))
            filler.append(b_group(0, 3) + (("B0hi", 0), 12500.0))
            for mtl in range(4):
                filler.append(c_group(0, mtl) + (
                    (("C", 0),) if mtl == 3 else (None,)) + (10200.0,))

            # everything else rides the filler queue, FIFO-ordered so
            # earlier-needed work is popped first.
            # ready = rough DMA arrival estimate of xt[Q] (ns)
            XT_READY = xt_ready or {1: 12000.0, 2: 16000.0, 3: 19000.0}
            for Q in range(1, NQ):
                rdy = XT_READY[Q]
                filler.append(b_group(Q, 0) + (None, rdy))
                filler.append(b_group(Q, 1) + (None, rdy))
                filler.append(b_group(Q, 2) + (None, rdy))
                filler.append(b_group(Q, 3) + (("B", Q), rdy))
                filler.append(c_group(Q, 0) + (None, rdy))
                filler.append(c_group(Q, 1) + (None, rdy))
                filler.append(c_group(Q, 2) + (None, rdy))
                filler.append(c_group(Q, 3) + (("C", Q), rdy))

            attn_quarter(0)
            filler += [tre_group(0, m) + (None, clk["pe"] + tre_ready[0]) for m in range(4)]
            attn_quarter(1)
            filler += [tre_group(1, m) + (None, clk["pe"] + tre_ready[1]) for m in range(4)]
            attn_quarter(2)
            filler += [tre_group(2, m) + (None, clk["pe"] + tre_ready[2]) for m in range(4)]
            attn_quarter(3)
            while filler:   # hard drain: ready-gating no longer applies
                pop_one()
            # batched final TRE: the kt=1 transposes+evicts first (their
            # norm waits pipeline on DVE/ACT), then all projections+stores.
            for m in range(4):
                pst = psA.tile([128, 128], BF16, name="ps_tr", tag="psA")
                nc.tensor.transpose(
                    pst[:], y_norm[(3, m)][:, 128:256], ident[:])
                dst = yT[(3,)][:, 512 + m * 128: 512 + (m + 1) * 128]
                if m % 2 == 0:
                    nc.scalar.copy(dst, pst[:])
                else:
                    nc.vector.tensor_copy(dst, pst[:])
            for m in range(4):
                mt = 12 + m
                o = outp.tile([128, 1024], BF16, name="o_t", tag="o_t")
                # exp is done -- reuse the (2-bank) psE tiles so the last
                # four projections pipeline instead of serializing on psA
                ps = psE.tile([128, 1024], F32, name="ps_a", tag="psE")
                for nch in range(2):
                    for kt in range(2):
                        nc.tensor.matmul(
                            ps[:, nch * 512:(nch + 1) * 512],
                            yT[(3,)][:, kt * 512 + m * 128:
                                     kt * 512 + (m + 1) * 128],
                            wp_sb[:, kt * 1024 + nch * 512:
                                  kt * 1024 + (nch + 1) * 512],
                            start=(kt == 0), stop=(kt == 1),
                        )
                    if nch == 0:
                        nc.scalar.copy(o[:, nch * 512:(nch + 1) * 512],
                                       ps[:, nch * 512:(nch + 1) * 512])
                    else:
                        nc.vector.tensor_copy(
                            o[:, nch * 512:(nch + 1) * 512],
                            ps[:, nch * 512:(nch + 1) * 512])
                nc.sync.dma_start(
                    out=out_d[mt * 128:(mt + 1) * 128, :], in_=o[:])

    if split_waits:
        _split_matmul_waits(nc)
    nc._stage_registry = stage_registry
    return nc


def _split_matmul_waits(nc):
    """Walrus codegen in this pipeline allows only one sync wait per
    instruction for most ISA structs (S3_LW, PSEUDO_DMA_DIRECT2D, S3D3_TS,
    ...). Move extra waits onto inserted NoOps on the same engine (program
    order preserves semantics)."""
    n_split = 0
    for bb in nc.main_func.blocks:
        out = []
        for ins in bb.instructions:
            si = getattr(ins, "sync_info", None)
            if (si is not None and len(si.on_wait) >= 2
                    and type(ins).__name__ != "InstNoOp"):
                for w in si.on_wait[:-1]:
                    nop = mybir.InstNoOp(name=f"I-wsplit-{nc.next_id()}",
                                         ins=[], outs=[])
                    nop.engine = ins.engine
                    nop.sync_info = mybir.SyncInfo(on_wait=[w], on_update=[])
                    out.append(nop)
                    n_split += 1
                ins.sync_info = mybir.SyncInfo(
                    on_wait=[si.on_wait[-1]], on_update=si.on_update)
            out.append(ins)
        bb.instructions[:] = out
    return n_split


def _bf16(a):
    import ml_dtypes
    return np.ascontiguousarray(a.astype(ml_dtypes.bfloat16))


def _fp8_hl(a):
    """Split f32 array into (hi, lo) fp8e4m3 planes with hi+lo ~= a."""
    import ml_dtypes
    hi = a.astype(ml_dtypes.float8_e4m3)
    lo = (a - hi.astype(np.float32)).astype(ml_dtypes.float8_e4m3)
    return hi, lo


def shard_inputs(x, Wqkv, bqkv, Wproj, bproj):
    x = np.asarray(x, np.float32)
    Wqkv = np.asarray(Wqkv, np.float32)
    bqkv = np.asarray(bqkv, np.float32)
    Wproj = np.asarray(Wproj, np.float32)
    in_maps = []
    xt_b = []
    import ml_dtypes
    for b in range(B):
        # xt[Q*128+p, half*4096 + kt*512 + m] = hl(x[b][Q*512+m, kt*128+p])
        xT = x[b].T  # [C, T]
        xt = xT.reshape(NKT, 128, NQ, 512).transpose(2, 1, 0, 3).reshape(
            NQ * 128, 4096)
        hi, lo = _fp8_hl(xt)
        xt2 = np.concatenate(
            [hi.reshape(NQ * 128, 4096), lo.reshape(NQ * 128, 4096)], axis=1
        ).reshape(NQ * 128, 2, 4096)
        # rows are Q-tiles of 128; halves interleave per Q-tile row block:
        # [Q*128+p, half*4096 + km] already correct since reshape keeps rows.
        xt_b.append(np.ascontiguousarray(xt2.reshape(NQ * 128, 8192)))
    for c in range(N_CORES):
        b, hg = c // 4, c % 4
        wqk = np.concatenate(
            [Wqkv[:, hg * 256:(hg + 1) * 256],
             Wqkv[:, C + hg * 256: C + (hg + 1) * 256]], axis=1) * WSCALE
        # [128, half*4096 + pc*1024 + kt*128 + m], half 0=lo 1=hi,
        # p stored in order (0,2,1,3)
        wqk2 = wqk.reshape(NKT, 128, 4, 128).transpose(1, 2, 0, 3)[
            :, [0, 2, 1, 3]].reshape(128, 4096)
        hi, lo = _fp8_hl(wqk2)
        wqk3 = np.concatenate([lo, hi], axis=1)
        bqk = np.concatenate(
            [bqkv[hg * 256:(hg + 1) * 256],
             bqkv[C + hg * 256: C + (hg + 1) * 256]]) * WSCALE  # [512]
        bqk2 = np.ascontiguousarray(bqk.reshape(4, 128).T)  # [128, 4]
        wv = np.zeros((C, 260), np.float32)
        wvl = np.zeros((1, 260), np.float32)
        for s in range(4):
            h = 4 * hg + s
            wv[:, s * 65:s * 65 + 64] = Wqkv[:, 2 * C + h * 64: 2 * C + (h + 1) * 64]
            wvl[0, s * 65:s * 65 + 64] = bqkv[2 * C + h * 64: 2 * C + (h + 1) * 64]
            wvl[0, s * 65 + 64] = 1.0
        wv *= WSCALE
        wvl *= WSCALE
        wv2 = wv.reshape(NKT, 128, 260).transpose(1, 0, 2).reshape(128, 8 * 260)
        hi, lo = _fp8_hl(wv2)
        # [128, half*2080 + kt*260 + n], half 0=lo 1=hi
        wv3 = np.concatenate(
            [lo.reshape(128, 2080), hi.reshape(128, 2080)], axis=1)
        wp = Wproj[hg * 256:(hg + 1) * 256, :]  # [256, C]
        wp2 = wp.reshape(2, 128, C).transpose(1, 0, 2).reshape(128, 2048)
        in_maps.append({
            "xt": xt_b[b],
            "wqk": np.ascontiguousarray(wqk3),
            "bqk": np.ascontiguousarray(bqk2, dtype=np.float32),
            "wv": np.ascontiguousarray(wv3),
            "wvl": _bf16(wvl),
            "wp": _bf16(wp2),
        })
    return in_maps


_NC_CACHE = {}


def kernel(x, Wqkv, bqkv, Wproj, bproj):
    from concourse.bass_utils import run_bass_kernel_spmd

    if "nc" not in _NC_CACHE:
        _NC_CACHE["nc"] = build_nc()
    nc = _NC_CACHE["nc"]
    in_maps = shard_inputs(x, Wqkv, bqkv, Wproj, bproj)
    res = run_bass_kernel_spmd(nc, in_maps, list(range(N_CORES)))
    _NC_CACHE["last_exec_time_ns"] = res.exec_time_ns
    bproj = np.asarray(bproj, np.float32)
    out = np.zeros((B, T, C), np.float32)
    for c in range(N_CORES):
        out[c // 4] += np.asarray(res.results[c]["out"], np.float32)
    out += bproj[None, None, :]
    return out



# revision 31
# speedup vs baseline: 1.0580x; 1.0024x over previous
"""Causal self-attention (B=2, T=2048, C=1024, NH=16, HD=64) on 8 NeuronCores.

Sharding: core c -> (batch b = c//4, head-group hg = c%4 of 4 heads).
Each core computes the qkv projection for its 4 heads from x[b], attention
for its 4 (b,h) units, and a partial output projection (row-parallel over
the head dim). Unshard = sum of the 4 partials per batch + bproj (host).

v2 design (all matmul operands bf16, PSUM accumulation f32):
  - Host pre-transposes x and pre-swizzles every weight into the exact
    SBUF layout, so DMAs are plain [128, W] copies and the device does no
    layout work at all (the old on-PE x-transpose stage is gone).
  - B: qkT [512, T] = wqk.T @ xT + bqk, evicted per quarter as bf16.
  - C: v_aug [T, 260] = [x | 1] @ wv_aug; per head 64 v columns + a ones
    column so the softmax row-sums fall out of the AV matmul for free.
  - D: att^T[j,i] per (head, i-chunk of 512) on PE (K=64); causal mask is
    a constant [128,128] lower-triangle(-8000) tile ACCUMULATED into the
    diagonal att psum before exp (no vector-engine masking). exp on ACT
    (scale 1/8, no max subtraction -- logits are O(1) by construction),
    output bf16.
  - AV flipped: y[i-tile, 4*65] accumulated as e_block.T @ v (e is the
    stationary operand), so each j-tile costs 65 moving rows instead of
    128+. Row 64 of each head's 65-col group = softmax sum S (from the
    ones column). Normalize with per-partition scalars (reciprocal + DVE
    tensor_scalar), transpose y on PE, project: out = yT.T @ wp (bf16
    partial, no bias; host adds bproj once).
  - The emission order software-pipelines ACT(exp) against PE: attention
    stages for quarter ic interleave with B/C work of quarter ic+1 and
    the transpose+projection of quarter ic-1 via a filler queue.
"""
import os
import sys

import numpy as np

for _p in ("/opt/trn_rl_repo",):
    if _p not in sys.path and os.path.isdir(_p):
        sys.path.insert(0, _p)

import concourse.bass as bass
import concourse.mybir as mybir
import concourse.tile as tile
from concourse.masks import make_identity

B, T, C, NH, HD = 2, 2048, 1024, 16, 64
F32 = mybir.dt.float32
BF16 = mybir.dt.bfloat16
FP8 = mybir.dt.float8e4
N_CORES = 8
NQ = 4          # token quarters (512 tokens each)
NKT = C // 128  # 8 contraction tiles
NT = T // 128   # 16 token tiles

# w (qkv) host-prescale: lifts fp8 hi/lo residuals of the N(0, 1/C)
# weights above the e4m3 subnormal floor. q,k,v come out 32x larger;
# the exp scale absorbs 32*32 for qk, and the v ones-column (also
# scaled) makes the softmax division self-normalizing.
WSCALE = 32.0
MASK_VAL = -8000.0 * WSCALE * WSCALE

# cost-model estimates (ns) used only to balance the filler interleave
MM_NS = 512 * 0.4167          # 512-row bf16 matmul
DRB_NS = 12 * 512 * 0.5 * 0.4167   # one B group: 12 DoubleRow matmuls
DRC_NS = (12 * 260 * 0.5 + 260) * 0.4167  # one C group incl bf16 bias mm
EXP_INIT_NS = 160.0           # per-exp-instruction access overhead
EXP_PAIR_NS = 1024 * 0.833 + EXP_INIT_NS
N_WARMUP = 9                  # f32 128-col warmup matmuls (PE clock ramp)


# measured-feedback filler pops (stage_seq_id -> ns), from iterating
# TimelineSim: simulate, map PE stalls to emission points, re-pop there.
_STAGE_HINTS = {2: 1602.0, 63: 718.0}


def build_nc(split_waits=True, hints=None, n_warmup=None,
             tre_ready=(1500.0, 20000.0, 26000.0), xt_ready=None):
    # hints: {stage_seq_id: extra_filler_ns} -- measured-feedback pops
    hints = _STAGE_HINTS if hints is None else hints
    N_WARMUP = n_warmup if n_warmup is not None else globals()["N_WARMUP"]
    stage_registry = []   # (stage_seq_id, first_inst_num) in emission order
    nc = bass.Bass()
    # fp8 hi/lo pair layouts (half-major so hi planes can DMA first; wqk
    # stores p in order (0,2,1,3) so the heads-0/1 q+k planes are the
    # leading 2KB of each half -- one DMA each):
    #   xt : [NQ*128, half*4096 + kt*512 + m]          half 0=hi 1=lo
    #   wqk: [128, half*4096 + pc*1024 + kt*128 + m]   half 0=lo 1=hi
    #   wv : [128, half*2080 + kt*260 + n]             half 0=lo 1=hi
    xt_d = nc.declare_dram_parameter("xt", [NQ * 128, 8192], FP8, isOutput=False)
    wqk_d = nc.declare_dram_parameter("wqk", [128, 8192], FP8, isOutput=False)
    bqk_d = nc.declare_dram_parameter("bqk", [128, 4], F32, isOutput=False)
    wv_d = nc.declare_dram_parameter("wv", [128, 2 * 8 * 260], FP8, isOutput=False)
    wvl_d = nc.declare_dram_parameter("wvl", [1, 260], BF16, isOutput=False)
    wp_d = nc.declare_dram_parameter("wp", [128, 2048], BF16, isOutput=False)
    out_d = nc.declare_dram_parameter("out", [T, C], BF16, isOutput=True)
    DR = mybir.MatmulPerfMode.DoubleRow

    with tile.TileContext(nc) as tc:
        with (
            tc.tile_pool(name="const", bufs=1) as const,
            tc.tile_pool(name="wts", bufs=1) as wts,
            tc.tile_pool(name="xtp", bufs=1) as xtp,
            tc.tile_pool(name="qkt", bufs=1) as qkt,
            tc.tile_pool(name="vsb", bufs=1) as vsb,
            tc.tile_pool(name="ep", bufs=12) as ep,
            tc.tile_pool(name="rsp", bufs=4) as rsp,
            tc.tile_pool(name="ynp", bufs=1) as ynp,
            tc.tile_pool(name="ytp", bufs=1) as ytp,
            tc.tile_pool(name="outp", bufs=16) as outp,
            tc.tile_pool(name="psA", bufs=2, space="PSUM") as psA,
            tc.tile_pool(name="psE", bufs=2, space="PSUM") as psE,
            tc.tile_pool(name="psY", bufs=2, space="PSUM") as psY,
        ):
            # ---- first DMAs on the critical path: wqk + x quarter 0.
            # Half-major fp8 layout: hi planes stream first so B(0)'s main
            # (hi*hi) matmuls can start ~1.2us in; lo planes follow and the
            # cross terms accumulate into the same psum group before stop.
            # warmup operand: DVE memset so the PE can start ~0.4us in
            wu = const.tile([128, 128], F32, name="wu")
            nc.vector.memset(wu[:], 0.25)
            xt_sb = [None] * NQ
            xt_sb[0] = xtp.tile([128, 8192], FP8, name="xt0", tag="xt0")
            wqk_sb = wts.tile([128, 8192], FP8, name="wqk_sb")
            nc.sync.dma_start(out=wqk_sb[:, 4096:6144],
                              in_=wqk_d[:, 4096:6144])  # p0+p2 hi (heads 0,1)
            nc.sync.dma_start(out=xt_sb[0][:, :2048], in_=xt_d[:128, :2048])
            nc.sync.dma_start(out=xt_sb[0][:, 2048:4096],
                              in_=xt_d[:128, 2048:4096])  # xt0 hi
            nc.sync.dma_start(out=wqk_sb[:, :2048],
                              in_=wqk_d[:, :2048])  # p0+p2 lo
            nc.sync.dma_start(out=xt_sb[0][:, 4096:6144],
                              in_=xt_d[:128, 4096:6144])
            nc.sync.dma_start(out=xt_sb[0][:, 6144:8192],
                              in_=xt_d[:128, 6144:8192])  # xt0 lo
            wv_sb = wts.tile([128, 2 * 8 * 260], FP8, name="wv_sb")
            nc.sync.dma_start(out=wv_sb[:], in_=wv_d[:])
            wvl_sb = wts.tile([1, 260], BF16, name="wvl_sb")
            nc.sync.dma_start(out=wvl_sb[:], in_=wvl_d[:])
            nc.sync.dma_start(out=wqk_sb[:, 6144:8192],
                              in_=wqk_d[:, 6144:8192])  # p1+p3 hi
            nc.sync.dma_start(out=wqk_sb[:, 2048:4096],
                              in_=wqk_d[:, 2048:4096])  # p1+p3 lo
            bqk_sb = const.tile([128, 4], F32, name="bqk_sb")
            nc.sync.dma_start(out=bqk_sb[:], in_=bqk_d[:])
            for q in range(1, NQ):
                xt_sb[q] = xtp.tile([128, 8192], FP8, name=f"xt{q}", tag=f"xt{q}")
            nc.sync.dma_start(out=xt_sb[1][:], in_=xt_d[128:256, :])
            wp_sb = wts.tile([128, 2048], BF16, name="wp_sb")
            nc.sync.dma_start(out=wp_sb[:], in_=wp_d[:])
            nc.sync.dma_start(out=xt_sb[2][:], in_=xt_d[256:384, :])
            nc.sync.dma_start(out=xt_sb[3][:], in_=xt_d[384:512, :])

            # ---- PE warmup: the cost model ramps the PE to full clock only
            # after ~3us of continuous busy. Burn f32 matmuls on a memset
            # tile during the DMA lead-in so real work starts at full speed.
            psW = psA.tile([128, 128], F32, name="ps_w", tag="psA")
            for r in range(N_WARMUP):
                nc.tensor.matmul(psW[:], wu[:], wu[:],
                                 start=(r == 0), stop=(r == N_WARMUP - 1))

            # ---- constants ----
            # gpsimd can't write bf16; build f32 then DVE copy-cast.
            ident32 = const.tile([128, 128], F32, name="ident32")
            make_identity(nc, ident32)
            ident = const.tile([128, 128], BF16, name="ident")
            nc.vector.tensor_copy(ident[:], ident32[:])
            # maskT[a, b] = 0 where a >= b else MASK_VAL; used as lhsT so the
            # psum receives M[j, i] = maskT[i, j] = 0 iff i >= j.
            maskf32 = const.tile([128, 128], F32, name="maskf32")
            nc.gpsimd.memset(maskf32[:], 0.0)
            nc.gpsimd.affine_select(
                out=maskf32[:], in_=maskf32[:],
                compare_op=mybir.AluOpType.is_ge, fill=MASK_VAL,
                base=0, channel_multiplier=1, pattern=[[-1, 128]],
            )
            maskT = const.tile([128, 128], BF16, name="maskT")
            nc.vector.tensor_copy(maskT[:], maskf32[:])
            ones32 = const.tile([1, 128], F32, name="ones32")
            nc.gpsimd.memset(ones32[:], 1.0)
            ones_b = const.tile([1, 128], BF16, name="ones_b")
            nc.vector.tensor_copy(ones_b[:], ones32[:])

            # ---- fp8 pair access-pattern helpers ----
            PCOL = {0: 0, 2: 1, 1: 2, 3: 3}   # p -> column block in wqk

            def wqk_main(p, kp):
                # (w_hi[2kp], w_hi[2kp+1])  [128, 2, 128]
                off = 4096 + PCOL[p] * 1024 + (2 * kp) * 128
                return wqk_sb[:, off:off + 256].rearrange(
                    "a (two m) -> a two m", two=2)

            def wqk_cross(p, kt):
                # (w_lo[kt], w_hi[kt])  dim1 stride 4096; pairs with
                # xt_cross's (x_hi, x_lo) -> lo*hi + hi*lo
                return wqk_sb[:].rearrange(
                    "a (h pkm) -> a h pkm", h=2)[
                        :, :, PCOL[p] * 1024 + kt * 128:
                        PCOL[p] * 1024 + (kt + 1) * 128]

            def xt_main(Q, kp, lo=0, n=512):
                # (x_hi[2kp], x_hi[2kp+1])  [128, 2, n]
                return xt_sb[Q][:].rearrange(
                    "a (h kt m) -> a h kt m", h=2, kt=8)[
                        :, 0, 2 * kp:2 * kp + 2, lo:lo + n]

            def xt_cross(Q, kt, lo=0, n=512):
                # (x_hi[kt], x_lo[kt])  dim1 stride 4096
                return xt_sb[Q][:].rearrange(
                    "a (h km) -> a h km", h=2)[
                        :, :, kt * 512 + lo:kt * 512 + lo + n]

            def wv_main(kp):
                off = 2080 + (2 * kp) * 260
                return wv_sb[:, off:off + 520].rearrange(
                    "a (two m) -> a two m", two=2)

            def wv_cross(kt):
                # (wv_lo[kt], wv_hi[kt])  dim1 stride 2080
                return wv_sb[:].rearrange(
                    "a (h kn) -> a h kn", h=2)[:, :, kt * 260:(kt + 1) * 260]

            # ---- persistent activations ----
            qkT = [qkt.tile([128, T], BF16, name=f"qkT{p}", tag=f"qkT{p}")
                   for p in range(4)]
            v_sb = [vsb.tile([128, 260], BF16, name=f"v{jt}", tag=f"v{jt}")
                    for jt in range(NT)]

            # ---------------- emission units ----------------
            # filler units: (est_pe_ns, closure). Emitted between attention
            # stages to keep PE busy while ACT chews exp.
            def b_group(Q, p):
                # qkT[p, Q] = (w_hi+w_lo).T (x_hi+x_lo) - w_lo.T x_lo over
                # K=1024: 4 DoubleRow mains (kt pairs, hi*hi) + 8 crosses.
                def emit():
                    ps = psA.tile([128, 512], F32, name="ps_qk", tag="psA")
                    for kp in range(4):
                        nc.tensor.matmul(
                            ps[:], wqk_main(p, kp), xt_main(Q, kp),
                            start=(kp == 0), stop=False, perf_mode=DR)
                    for kt in range(NKT):
                        nc.tensor.matmul(
                            ps[:], wqk_cross(p, kt), xt_cross(Q, kt),
                            start=False, stop=(kt == NKT - 1), perf_mode=DR)
                    nc.vector.tensor_scalar_add(
                        qkT[p][:, Q * 512:(Q + 1) * 512], ps[:],
                        bqk_sb[:, p:p + 1])
                return (DRB_NS, emit)

            def c_group(Q, mtl):
                def emit():
                    jt = 4 * Q + mtl
                    ps = psA.tile([128, 260], F32, name="ps_v", tag="psA")
                    for kp in range(4):
                        nc.tensor.matmul(
                            ps[:], xt_main(Q, kp, lo=mtl * 128, n=128),
                            wv_main(kp),
                            start=(kp == 0), stop=False, perf_mode=DR)
                    for kt in range(NKT):
                        nc.tensor.matmul(
                            ps[:], xt_cross(Q, kt, lo=mtl * 128, n=128),
                            wv_cross(kt),
                            start=False, stop=False, perf_mode=DR)
                    nc.tensor.matmul(ps[:], ones_b[:], wvl_sb[:],
                                     start=False, stop=True)
                    nc.vector.tensor_copy(v_sb[jt][:], ps[:])
                return (DRC_NS, emit)

            y_norm = {}   # (ic, itl) -> tile
            yT = {}       # (ic, kt) -> tile

            def tre_group(ic, mtl, tail=False):
                """Transpose y_norm[ic, mtl] into yT and project+store.
                yT[(ic,)] is one [128, 1024] tile: kt block at col kt*512.
                tail=True spreads evictions across DVE and ACT (end of
                kernel, ACT is idle)."""
                def emit():
                    mt = 4 * ic + mtl
                    pst = psA.tile([128, 256], BF16, name="ps_tr", tag="psA")
                    for kt in range(2):
                        nc.tensor.transpose(
                            pst[:, kt * 128:(kt + 1) * 128],
                            y_norm[(ic, mtl)][:, kt * 128:(kt + 1) * 128],
                            ident[:])
                    # one strided evict writes both kt blocks of yT
                    dst = yT[(ic,)][:].rearrange(
                        "p (a b) -> p a b", a=2)[:, :, mtl * 128:(mtl + 1) * 128]
                    if tail:
                        nc.scalar.copy(dst, pst[:])
                    else:
                        nc.vector.tensor_copy(dst, pst[:])
                    o = outp.tile([128, 1024], BF16, name="o_t", tag="o_t")
                    for nch in range(2):
                        ps = psA.tile([128, 512], F32, name="ps_o", tag="psA")
                        for kt in range(2):
                            nc.tensor.matmul(
                                ps[:],
                                yT[(ic,)][:, kt * 512 + mtl * 128:
                                          kt * 512 + (mtl + 1) * 128],
                                wp_sb[:, kt * 1024 + nch * 512: kt * 1024 + (nch + 1) * 512],
                                start=(kt == 0), stop=(kt == 1),
                            )
                        if tail and nch == 0:
                            nc.scalar.copy(o[:, nch * 512:(nch + 1) * 512], ps[:])
                        else:
                            nc.vector.tensor_copy(
                                o[:, nch * 512:(nch + 1) * 512], ps[:])
                    # one store per m-tile: each DMA costs ~650ns of
                    # serialized SP/HWDGE issue regardless of size
                    nc.sync.dma_start(
                        out=out_d[mt * 128:(mt + 1) * 128, :], in_=o[:])
                return (2 * 128 * 0.4167 + 4 * MM_NS, emit)

            def itl_key(mtl):
                return mtl

            # global filler deque: (pe_ns, emit, marker). markers order
            # dependencies: ("B", ic) must emit before attn(ic)'s att reads
            # qkT; ("C", ic) before attn(ic)'s diagonal AVs read v.
            filler = []
            consumed = {("B", 0)}
            # global emission clock (ns estimates): pe = PE busy frontier,
            # act = ACT (exp) completion frontier. Used to decide when PE
            # needs filler so it never idles waiting for exp.
            clk = {"pe": 0.0, "act": 0.0}
            SEM_LAT = 100.0
            tail_mode = [False]

            def pop_one():
                pe_ns, emit, marker, ready = filler.pop(0)
                emit()
                if marker:
                    consumed.add(marker)
                clk["pe"] += pe_ns

            def pop_filler(need_pe_ns, force=False):
                got = 0.0
                while filler and got < need_pe_ns:
                    if not force and filler[0][3] > clk["pe"]:
                        break  # head unit's inputs not DMA'd yet
                    pe_ns = filler[0][0]
                    pop_one()
                    got += pe_ns
                return got

            def advance_pe_to(t):
                """PE must reach estimated time t before the next emitted
                instruction can run: emit filler to cover the wait."""
                while clk["pe"] < t and filler and filler[0][3] <= clk["pe"]:
                    pop_one()
                if clk["pe"] < t:
                    clk["pe"] = t  # PE idles

            def force_pop(marker):
                while marker not in consumed and filler:
                    pop_one()

            # ---------------- attention head ----------------
            LAG = 3

            def atthead(s, ic, prev_fin=None, prev_fin_ready=0.0):
                if ic == 0 and s == 2:
                    # heads 2,3 read qkT p1/p3, which ride the filler
                    force_pop(("B0hi", 0))
                qrow = (s % 2) * 64
                qtile = qkT[s // 2]
                ktile = qkT[2 + s // 2]
                ps_y = psY.tile([128, 260], F32, name="ps_y", tag="psY")
                n_av = [0] * 4            # AVs emitted per i-tile region
                tot_av = [4 * ic + itl + 1 for itl in range(4)]
                # jt -> (e tile, col of i-block 0); filled as stages emit
                e_of = {}

                def av(itl, jt):
                    # PSUM groups are bank(2KB)-granular: regions of ps_y
                    # accumulate strictly one group at a time (r0 rolls with
                    # the stages; r1..r3 burst after all e tiles exist).
                    if jt >= 4 * ic:
                        force_pop(("C", ic))
                    e, base = e_of[jt]
                    nc.tensor.matmul(
                        ps_y[:, itl * 65:itl * 65 + 65],
                        e[:, base + itl * 128: base + itl * 128 + 128],
                        v_sb[jt][:, s * 65:s * 65 + 65],
                        start=(n_av[itl] == 0),
                        stop=(n_av[itl] == tot_av[itl] - 1),
                    )
                    n_av[itl] += 1

                # stage list: full pairs first, then the diagonal halves.
                att_stages = []

                for pr in range(2 * ic):
                    def mk_att(pr=pr):
                        ps_a = psE.tile([128, 1024], F32, name="ps_a", tag="psE")
                        e = ep.tile([128, 1024], BF16, name="e_t", tag="e_t")
                        for h in range(2):
                            jt = 2 * pr + h
                            nc.tensor.matmul(
                                ps_a[:, h * 512:(h + 1) * 512],
                                ktile[qrow:qrow + 64, jt * 128:(jt + 1) * 128],
                                qtile[qrow:qrow + 64, ic * 512:(ic + 1) * 512],
                                start=True, stop=True,
                            )
                            e_of[jt] = (e, h * 512)
                        nc.scalar.activation(
                            e[:], ps_a[:], mybir.ActivationFunctionType.Exp,
                            scale=float(HD) ** -0.5 / (WSCALE * WSCALE))
                    # stage jts, ACT ns, PE ns
                    att_stages.append((mk_att, [2 * pr, 2 * pr + 1],
                                       EXP_PAIR_NS, 2 * MM_NS))

                for half in range(2):
                    def mk_att(half=half):
                        ps_a = psE.tile([128, 1024], F32, name="ps_a", tag="psE")
                        e = ep.tile([128, 1024], BF16, name="e_t", tag="e_t")
                        for h in range(2):
                            o = (2 * half + h) * 128
                            jt = 4 * ic + 2 * half + h
                            # diagonal 128-col block: att + mask accumulated
                            nc.tensor.matmul(
                                ps_a[:, h * 512 + o: h * 512 + o + 128],
                                ktile[qrow:qrow + 64, jt * 128:(jt + 1) * 128],
                                qtile[qrow:qrow + 64,
                                      ic * 512 + o: ic * 512 + o + 128],
                                start=True, stop=False,
                            )
                            nc.tensor.matmul(
                                ps_a[:, h * 512 + o: h * 512 + o + 128],
                                maskT[:], ident[:],
                                start=False, stop=True,
                            )
                            # unmasked remainder of the row band
                            if o + 128 < 512:
                                nc.tensor.matmul(
                                    ps_a[:, h * 512 + o + 128:(h + 1) * 512],
                                    ktile[qrow:qrow + 64, jt * 128:(jt + 1) * 128],
                                    qtile[qrow:qrow + 64,
                                          ic * 512 + o + 128:(ic + 1) * 512],
                                    start=True, stop=True,
                                )
                            e_of[jt] = (e, h * 512)
                            nc.scalar.activation(
                                e[:, h * 512 + o:(h + 1) * 512],
                                ps_a[:, h * 512 + o:(h + 1) * 512],
                                mybir.ActivationFunctionType.Exp,
                                scale=float(HD) ** -0.5 / (WSCALE * WSCALE))
                    jts = [4 * ic + 2 * half, 4 * ic + 2 * half + 1]
                    att_pe = (896 - 512 * half) * 0.4167 + 2 * 128 * 0.4167
                    exp_ns = (896 - 512 * half) * 0.833 + 2 * EXP_INIT_NS
                    att_stages.append((mk_att, jts, exp_ns, att_pe))

                # emit: stages; region 0's group rolls along (lagged);
                # regions 1..3 burst after the last stage. The global clock
                # models the psE ring (bufs=2): att stage k reuses stage
                # k-2's psum, so PE must not reach att(k) before exp(k-2)
                # completes -- filler covers the difference.
                AV_NS = 65 * 0.4167
                done = []       # per stage: jts whose e exists
                exp_done = []   # per stage: est. exp completion time
                fin_emitted = [prev_fin is None]
                for k, (mk, jts, exp_ns, att_pe) in enumerate(att_stages):
                    sid = len(stage_registry)
                    stage_registry.append((sid, nc.next_id()))
                    h = hints.get(sid, 0.0)
                    if h > 0:
                        pop_filler(h, force=True)
                    if k >= 2:
                        advance_pe_to(exp_done[k - 2])
                    mk()
                    clk["pe"] += att_pe
                    clk["act"] = max(clk["act"],
                                     clk["pe"] + SEM_LAT) + exp_ns
                    exp_done.append(clk["act"] + SEM_LAT)
                    done.append(jts)
                    if k >= LAG:
                        for jt in done[k - LAG]:
                            if jt <= 4 * ic:
                                av(0, jt)
                                clk["pe"] += AV_NS
                    # previous head's deferred tail: emit once its exps are
                    # surely done (covered by our att stream, not filler)
                    if not fin_emitted[0] and (
                            clk["pe"] >= prev_fin_ready
                            or k >= len(att_stages) - 2):
                        prev_fin()
                        fin_emitted[0] = True
                if not fin_emitted[0]:
                    advance_pe_to(prev_fin_ready)
                    prev_fin()

                def fin():
                    # tail: r0 leftovers + r1 only need the half0 diag exp;
                    # r2/r3 additionally need half1 -- gate separately so
                    # the early regions stream while half1's exp finishes.
                    if len(exp_done) >= 2:
                        advance_pe_to(exp_done[-2])
                    for jts in done[max(0, len(done) - LAG):]:
                        for jt in jts:
                            if jt <= 4 * ic:
                                av(0, jt)
                                clk["pe"] += AV_NS
                    for jt in range(4 * ic + 2):
                        av(1, jt)
                        clk["pe"] += AV_NS
                    if exp_done:
                        advance_pe_to(exp_done[-1])
                    for itl in range(2, 4):
                        for jt in range(4 * ic + itl + 1):
                            av(itl, jt)
                            clk["pe"] += AV_NS

                    # normalize: rs = 1/S per i-tile, per-partition scalars
                    rs = rsp.tile([128, 4], F32, name="rs_t", tag="rs_t")
                    with nc.allow_low_precision(
                            reason="softmax sum reciprocal"):
                        nc.vector.reciprocal(rs[:], ps_y[:, 64::65])
                    for itl in range(4):
                        yn = y_norm.get((ic, itl))
                        if yn is None:
                            yn = ynp.tile([128, 256], BF16, name="y_n",
                                          tag=f"yn{(ic % 2) * 4 + itl}")
                            y_norm[(ic, itl)] = yn
                        if tail_mode[0] and itl % 2 == 1:
                            # end of kernel: ACT is idle, split the norms
                            nc.scalar.mul(
                                yn[:, s * 64:(s + 1) * 64],
                                ps_y[:, itl * 65:itl * 65 + 64],
                                rs[:, itl:itl + 1])
                        else:
                            nc.vector.tensor_scalar(
                                out=yn[:, s * 64:(s + 1) * 64],
                                in0=ps_y[:, itl * 65:itl * 65 + 64],
                                scalar1=rs[:, itl:itl + 1],
                                scalar2=None,
                                op0=mybir.AluOpType.mult,
                            )
                return fin, (exp_done[-2] if len(exp_done) >= 2 else 0.0)

            # ---------------- schedule ----------------
            def attn_quarter(ic):
                force_pop(("B", ic))
                yT[(ic,)] = ytp.tile([128, 1024], BF16, name="yT_t",
                                     tag=f"yT{ic % 2}")
                fin, ready = None, 0.0
                for s in range(4):
                    fin, ready = atthead(s, ic, fin, ready)
                    if ic == 3 and s == 2:
                        # fin(s=1) was just emitted inside atthead(2).
                        # yT's kt=0 half only reads heads 0,1 (cols 0..127
                        # of y_norm): transpose it now so it fills the last
                        # head's exp window and shortens the final tail.
                        for m in range(4):
                            pst = psA.tile([128, 128], BF16,
                                           name="ps_tr", tag="psA")
                            nc.tensor.transpose(
                                pst[:], y_norm[(3, m)][:, 0:128], ident[:])
                            nc.vector.tensor_copy(
                                yT[(3,)][:, m * 128:(m + 1) * 128], pst[:])
                        clk["pe"] += 4 * 128 * 0.4167
                # last head's tail (fin gates its own exp waits with filler)
                if ic == 3:
                    tail_mode[0] = True
                fin()

            # it0: B(0) p0/p2 staged by kt-halves so PE starts on the first
            # DMA arrivals. Heads 0,1 of attn(0) only need p0 (their q) and
            # p2 (their k), so attention -- and with it ACT's exp stream --
            # can start ~5us earlier; B(0) p1/p3 and C(0) ride the filler.
            # hi*hi mains first (only need the hi DMA planes), then the
            # cross terms once the lo planes land.
            psB0 = {}
            for p in (0, 2):
                psB0[p] = psA.tile([128, 512], F32, name="ps_qk", tag="psA")
                for kp in range(4):
                    nc.tensor.matmul(psB0[p][:], wqk_main(p, kp),
                                     xt_main(0, kp),
                                     start=(kp == 0), stop=False, perf_mode=DR)
            for p in (0, 2):
                for kt in range(NKT):
                    nc.tensor.matmul(
                        psB0[p][:], wqk_cross(p, kt), xt_cross(0, kt),
                        start=False, stop=(kt == NKT - 1), perf_mode=DR)
                nc.vector.tensor_scalar_add(
                    qkT[p][:, :512], psB0[p][:], bqk_sb[:, p:p + 1])
            # startup estimate: DMA lead-in + warmup + B(0) p0/p2
            clk["pe"] = 6000.0
            filler.append(b_group(0, 1) + (None, 9000.0))
            filler.append(b_group(0, 3) + (("B0hi", 0), 9700.0))
            for mtl in range(4):
                filler.append(c_group(0, mtl) + (
                    (("C", 0),) if mtl == 3 else (None,)) + (8300.0,))

            # everything else rides the filler queue, FIFO-ordered so
            # earlier-needed work is popped first.
            # ready = rough DMA arrival estimate of xt[Q] (ns)
            XT_READY = xt_ready or {1: 12700.0, 2: 17100.0, 3: 20000.0}
            for Q in range(1, NQ):
                rdy = XT_READY[Q]
                filler.append(b_group(Q, 0) + (None, rdy))
                filler.append(b_group(Q, 1) + (None, rdy))
                filler.append(b_group(Q, 2) + (None, rdy))
                filler.append(b_group(Q, 3) + (("B", Q), rdy))
                filler.append(c_group(Q, 0) + (None, rdy))
                filler.append(c_group(Q, 1) + (None, rdy))
                filler.append(c_group(Q, 2) + (None, rdy))
                filler.append(c_group(Q, 3) + (("C", Q), rdy))

            attn_quarter(0)
            filler += [tre_group(0, m) + (None, clk["pe"] + tre_ready[0]) for m in range(4)]
            attn_quarter(1)
            filler += [tre_group(1, m) + (None, clk["pe"] + tre_ready[1]) for m in range(4)]
            attn_quarter(2)
            filler += [tre_group(2, m) + (None, clk["pe"] + tre_ready[2]) for m in range(4)]
            attn_quarter(3)
            while filler:   # hard drain: ready-gating no longer applies
                pop_one()
            # batched final TRE: the kt=1 transposes+evicts first (their
            # norm waits pipeline on DVE/ACT), then all projections+stores.
            for m in range(4):
                pst = psA.tile([128, 128], BF16, name="ps_tr", tag="psA")
                nc.tensor.transpose(
                    pst[:], y_norm[(3, m)][:, 128:256], ident[:])
                dst = yT[(3,)][:, 512 + m * 128: 512 + (m + 1) * 128]
                if m % 2 == 0:
                    nc.scalar.copy(dst, pst[:])
                else:
                    nc.vector.tensor_copy(dst, pst[:])
            for m in range(4):
                mt = 12 + m
                o = outp.tile([128, 1024], BF16, name="o_t", tag="o_t")
                # exp is done -- reuse the (2-bank) psE tiles so the last
                # four projections pipeline instead of serializing on psA
                ps = psE.tile([128, 1024], F32, name="ps_a", tag="psE")
                for nch in range(2):
                    for kt in range(2):
                        nc.tensor.matmul(
                            ps[:, nch * 512:(nch + 1) * 512],
                            yT[(3,)][:, kt * 512 + m * 128:
                                     kt * 512 + (m + 1) * 128],
                            wp_sb[:, kt * 1024 + nch * 512:
                                  kt * 1024 + (nch + 1) * 512],
                            start=(kt == 0), stop=(kt == 1),
                        )
                    if nch == 0:
                        nc.scalar.copy(o[:, nch * 512:(nch + 1) * 512],
                                       ps[:, nch * 512:(nch + 1) * 512])
                    else:
                        nc.vector.tensor_copy(
                            o[:, nch * 512:(nch + 1) * 512],
                            ps[:, nch * 512:(nch + 1) * 512])
                nc.sync.dma_start(
                    out=out_d[mt * 128:(mt + 1) * 128, :], in_=o[:])

    if split_waits:
        _split_matmul_waits(nc)
    nc._stage_registry = stage_registry
    return nc


def _split_matmul_waits(nc):
    """Walrus codegen in this pipeline allows only one sync wait per
    instruction for most ISA structs (S3_LW, PSEUDO_DMA_DIRECT2D, S3D3_TS,
    ...). Move extra waits onto inserted NoOps on the same engine (program
    order preserves semantics)."""
    n_split = 0
    for bb in nc.main_func.blocks:
        out = []
        for ins in bb.instructions:
            si = getattr(ins, "sync_info", None)
            if (si is not None and len(si.on_wait) >= 2
                    and type(ins).__name__ != "InstNoOp"):
                for w in si.on_wait[:-1]:
                    nop = mybir.InstNoOp(name=f"I-wsplit-{nc.next_id()}",
                                         ins=[], outs=[])
                    nop.engine = ins.engine
                    nop.sync_info = mybir.SyncInfo(on_wait=[w], on_update=[])
                    out.append(nop)
                    n_split += 1
                ins.sync_info = mybir.SyncInfo(
                    on_wait=[si.on_wait[-1]], on_update=si.on_update)
            out.append(ins)
        bb.instructions[:] = out
    return n_split


def _bf16(a):
    import ml_dtypes
    return np.ascontiguousarray(a.astype(ml_dtypes.bfloat16))


def _fp8_hl(a):
    """Split f32 array into (hi, lo) fp8e4m3 planes with hi+lo ~= a."""
    import ml_dtypes
    hi = a.astype(ml_dtypes.float8_e4m3)
    lo = (a - hi.astype(np.float32)).astype(ml_dtypes.float8_e4m3)
    return hi, lo


def shard_inputs(x, Wqkv, bqkv, Wproj, bproj):
    x = np.asarray(x, np.float32)
    Wqkv = np.asarray(Wqkv, np.float32)
    bqkv = np.asarray(bqkv, np.float32)
    Wproj = np.asarray(Wproj, np.float32)
    in_maps = []
    xt_b = []
    import ml_dtypes
    for b in range(B):
        # xt[Q*128+p, half*4096 + kt*512 + m] = hl(x[b][Q*512+m, kt*128+p])
        xT = x[b].T  # [C, T]
        xt = xT.reshape(NKT, 128, NQ, 512).transpose(2, 1, 0, 3).reshape(
            NQ * 128, 4096)
        hi, lo = _fp8_hl(xt)
        xt2 = np.concatenate(
            [hi.reshape(NQ * 128, 4096), lo.reshape(NQ * 128, 4096)], axis=1
        ).reshape(NQ * 128, 2, 4096)
        # rows are Q-tiles of 128; halves interleave per Q-tile row block:
        # [Q*128+p, half*4096 + km] already correct since reshape keeps rows.
        xt_b.append(np.ascontiguousarray(xt2.reshape(NQ * 128, 8192)))
    for c in range(N_CORES):
        b, hg = c // 4, c % 4
        wqk = np.concatenate(
            [Wqkv[:, hg * 256:(hg + 1) * 256],
             Wqkv[:, C + hg * 256: C + (hg + 1) * 256]], axis=1) * WSCALE
        # [128, half*4096 + pc*1024 + kt*128 + m], half 0=lo 1=hi,
        # p stored in order (0,2,1,3)
        wqk2 = wqk.reshape(NKT, 128, 4, 128).transpose(1, 2, 0, 3)[
            :, [0, 2, 1, 3]].reshape(128, 4096)
        hi, lo = _fp8_hl(wqk2)
        wqk3 = np.concatenate([lo, hi], axis=1)
        bqk = np.concatenate(
            [bqkv[hg * 256:(hg + 1) * 256],
             bqkv[C + hg * 256: C + (hg + 1) * 256]]) * WSCALE  # [512]
        bqk2 = np.ascontiguousarray(bqk.reshape(4, 128).T)  # [128, 4]
        wv = np.zeros((C, 260), np.float32)
        wvl = np.zeros((1, 260), np.float32)
        for s in range(4):
            h = 4 * hg + s
            wv[:, s * 65:s * 65 + 64] = Wqkv[:, 2 * C + h * 64: 2 * C + (h + 1) * 64]
            wvl[0, s * 65:s * 65 + 64] = bqkv[2 * C + h * 64: 2 * C + (h + 1) * 64]
            wvl[0, s * 65 + 64] = 1.0
        wv *= WSCALE
        wvl *= WSCALE
        wv2 = wv.reshape(NKT, 128, 260).transpose(1, 0, 2).reshape(128, 8 * 260)
        hi, lo = _fp8_hl(wv2)
        # [128, half*2080 + kt*260 + n], half 0=lo 1=hi
        wv3 = np.concatenate(
            [lo.reshape(128, 2080), hi.reshape(128, 2080)], axis=1)
        wp = Wproj[hg * 256:(hg + 1) * 256, :]  # [256, C]
        wp2 = wp.reshape(2, 128, C).transpose(1, 0, 2).reshape(128, 2048)
        in_maps.append({
            "xt": xt_b[b],
            "wqk": np.ascontiguousarray(wqk3),
            "bqk": np.ascontiguousarray(bqk2, dtype=np.float32),
            "wv": np.ascontiguousarray(wv3),
            "wvl": _bf16(wvl),
            "wp": _bf16(wp2),
        })
    return in_maps


_NC_CACHE = {}


def kernel(x, Wqkv, bqkv, Wproj, bproj):
    from concourse.bass_utils import run_bass_kernel_spmd

    if "nc" not in _NC_CACHE:
        _NC_CACHE["nc"] = build_nc()
    nc = _NC_CACHE["nc"]
    in_maps = shard_inputs(x, Wqkv, bqkv, Wproj, bproj)
    res = run_bass_kernel_spmd(nc, in_maps, list(range(N_CORES)))
    _NC_CACHE["last_exec_time_ns"] = res.exec_time_ns
    bproj = np.asarray(bproj, np.float32)
    out = np.zeros((B, T, C), np.float32)
    for c in range(N_CORES):
        out[c // 4] += np.asarray(res.results[c]["out"], np.float32)
    out += bproj[None, None, :]
    return out



# revision 37
# speedup vs baseline: 1.0719x; 1.0132x over previous
"""Causal self-attention (B=2, T=2048, C=1024, NH=16, HD=64) on 8 NeuronCores.

Sharding: core c -> (batch b = c//4, head-group hg = c%4 of 4 heads).
Each core computes the qkv projection for its 4 heads from x[b], attention
for its 4 (b,h) units, and a partial output projection (row-parallel over
the head dim). Unshard = sum of the 4 partials per batch + bproj (host).

v2 design (all matmul operands bf16, PSUM accumulation f32):
  - Host pre-transposes x and pre-swizzles every weight into the exact
    SBUF layout, so DMAs are plain [128, W] copies and the device does no
    layout work at all (the old on-PE x-transpose stage is gone).
  - B: qkT [512, T] = wqk.T @ xT + bqk, evicted per quarter as bf16.
  - C: v_aug [T, 260] = [x | 1] @ wv_aug; per head 64 v columns + a ones
    column so the softmax row-sums fall out of the AV matmul for free.
  - D: att^T[j,i] per (head, i-chunk of 512) on PE (K=64); causal mask is
    a constant [128,128] lower-triangle(-8000) tile ACCUMULATED into the
    diagonal att psum before exp (no vector-engine masking). exp on ACT
    (scale 1/8, no max subtraction -- logits are O(1) by construction),
    output bf16.
  - AV flipped: y[i-tile, 4*65] accumulated as e_block.T @ v (e is the
    stationary operand), so each j-tile costs 65 moving rows instead of
    128+. Row 64 of each head's 65-col group = softmax sum S (from the
    ones column). Normalize with per-partition scalars (reciprocal + DVE
    tensor_scalar), transpose y on PE, project: out = yT.T @ wp (bf16
    partial, no bias; host adds bproj once).
  - The emission order software-pipelines ACT(exp) against PE: attention
    stages for quarter ic interleave with B/C work of quarter ic+1 and
    the transpose+projection of quarter ic-1 via a filler queue.
"""
import os
import sys

import numpy as np

for _p in ("/opt/trn_rl_repo",):
    if _p not in sys.path and os.path.isdir(_p):
        sys.path.insert(0, _p)

import concourse.bass as bass
import concourse.mybir as mybir
import concourse.tile as tile
from concourse.masks import make_identity

B, T, C, NH, HD = 2, 2048, 1024, 16, 64
F32 = mybir.dt.float32
BF16 = mybir.dt.bfloat16
FP8 = mybir.dt.float8e4
N_CORES = 8
NQ = 4          # token quarters (512 tokens each)
NKT = C // 128  # 8 contraction tiles
NT = T // 128   # 16 token tiles

# w (qkv) host-prescale: lifts fp8 hi/lo residuals of the N(0, 1/C)
# weights above the e4m3 subnormal floor. q,k,v come out 32x larger;
# the exp scale absorbs 32*32 for qk, and the v ones-column (also
# scaled) makes the softmax division self-normalizing.
WSCALE = 32.0
MASK_VAL = -8000.0 * WSCALE * WSCALE

# cost-model estimates (ns) used only to balance the filler interleave
MM_NS = 512 * 0.4167          # 512-row bf16 matmul
DRB_NS = 12 * 512 * 0.5 * 0.4167   # one B group: 12 DoubleRow matmuls
DRC_NS = (12 * 260 * 0.5 + 260) * 0.4167  # one C group incl bf16 bias mm
EXP_INIT_NS = 160.0           # per-exp-instruction access overhead
EXP_PAIR_NS = 1024 * 0.833 + EXP_INIT_NS
N_WARMUP = 9                  # f32 128-col warmup matmuls (PE clock ramp)


# measured-feedback filler pops (stage_seq_id -> ns), from iterating
# TimelineSim: simulate, map PE stalls to emission points, re-pop there.
_STAGE_HINTS = {2: 1602.0, 63: 718.0}


def build_nc(split_waits=True, hints=None, n_warmup=None,
             tre_ready=(44000.0, 42000.0, 30000.0), xt_ready=None):
    # hints: {stage_seq_id: extra_filler_ns} -- measured-feedback pops
    hints = _STAGE_HINTS if hints is None else hints
    N_WARMUP = n_warmup if n_warmup is not None else globals()["N_WARMUP"]
    stage_registry = []   # (stage_seq_id, first_inst_num) in emission order
    nc = bass.Bass()
    # fp8 hi/lo pair layouts (half-major so hi planes can DMA first; wqk
    # stores p in order (0,2,1,3) so the heads-0/1 q+k planes are the
    # leading 2KB of each half -- one DMA each):
    #   xt : [NQ*128, half*4096 + kt*512 + m]          half 0=hi 1=lo
    #   wqk: [128, half*4096 + pc*1024 + kt*128 + m]   half 0=lo 1=hi
    #   wv : [128, half*2080 + kt*260 + n]             half 0=lo 1=hi
    xt_d = nc.declare_dram_parameter("xt", [NQ * 128, 8192], FP8, isOutput=False)
    wqk_d = nc.declare_dram_parameter("wqk", [128, 8192], FP8, isOutput=False)
    bqk_d = nc.declare_dram_parameter("bqk", [128, 4], F32, isOutput=False)
    wv_d = nc.declare_dram_parameter("wv", [128, 2 * 8 * 260], FP8, isOutput=False)
    wvl_d = nc.declare_dram_parameter("wvl", [1, 260], BF16, isOutput=False)
    wp_d = nc.declare_dram_parameter("wp", [128, 2048], BF16, isOutput=False)
    out_d = nc.declare_dram_parameter("out", [T, C], BF16, isOutput=True)
    DR = mybir.MatmulPerfMode.DoubleRow

    with tile.TileContext(nc) as tc:
        with (
            tc.tile_pool(name="const", bufs=1) as const,
            tc.tile_pool(name="wts", bufs=1) as wts,
            tc.tile_pool(name="xtp", bufs=1) as xtp,
            tc.tile_pool(name="qkt", bufs=1) as qkt,
            tc.tile_pool(name="vsb", bufs=1) as vsb,
            tc.tile_pool(name="ep", bufs=12) as ep,
            tc.tile_pool(name="rsp", bufs=4) as rsp,
            tc.tile_pool(name="ynp", bufs=1) as ynp,
            tc.tile_pool(name="ytp", bufs=1) as ytp,
            tc.tile_pool(name="outp", bufs=16) as outp,
            tc.tile_pool(name="psA", bufs=2, space="PSUM") as psA,
            tc.tile_pool(name="psE", bufs=2, space="PSUM") as psE,
            tc.tile_pool(name="psY", bufs=2, space="PSUM") as psY,
        ):
            # ---- first DMAs on the critical path: wqk + x quarter 0.
            # Half-major fp8 layout: hi planes stream first so B(0)'s main
            # (hi*hi) matmuls can start ~1.2us in; lo planes follow and the
            # cross terms accumulate into the same psum group before stop.
            # warmup operand: DVE memset so the PE can start ~0.4us in
            wu = const.tile([128, 128], F32, name="wu")
            nc.gpsimd.memset(wu[:], 0.25)
            xt_sb = [None] * NQ
            xt_sb[0] = xtp.tile([128, 8192], FP8, name="xt0", tag="xt0")
            wqk_sb = wts.tile([128, 8192], FP8, name="wqk_sb")
            nc.sync.dma_start(out=wqk_sb[:, 4096:6144],
                              in_=wqk_d[:, 4096:6144])  # p0+p2 hi (heads 0,1)
            nc.sync.dma_start(out=xt_sb[0][:, :2048], in_=xt_d[:128, :2048])
            nc.sync.dma_start(out=xt_sb[0][:, 2048:4096],
                              in_=xt_d[:128, 2048:4096])  # xt0 hi
            nc.sync.dma_start(out=wqk_sb[:, :2048],
                              in_=wqk_d[:, :2048])  # p0+p2 lo
            nc.sync.dma_start(out=xt_sb[0][:, 4096:6144],
                              in_=xt_d[:128, 4096:6144])
            nc.sync.dma_start(out=xt_sb[0][:, 6144:8192],
                              in_=xt_d[:128, 6144:8192])  # xt0 lo
            wv_sb = wts.tile([128, 2 * 8 * 260], FP8, name="wv_sb")
            nc.sync.dma_start(out=wv_sb[:], in_=wv_d[:])
            wvl_sb = wts.tile([1, 260], BF16, name="wvl_sb")
            nc.sync.dma_start(out=wvl_sb[:], in_=wvl_d[:])
            nc.sync.dma_start(out=wqk_sb[:, 6144:8192],
                              in_=wqk_d[:, 6144:8192])  # p1+p3 hi
            nc.sync.dma_start(out=wqk_sb[:, 2048:4096],
                              in_=wqk_d[:, 2048:4096])  # p1+p3 lo
            bqk_sb = const.tile([128, 4], F32, name="bqk_sb")
            nc.sync.dma_start(out=bqk_sb[:], in_=bqk_d[:])
            for q in range(1, NQ):
                xt_sb[q] = xtp.tile([128, 8192], FP8, name=f"xt{q}", tag=f"xt{q}")
            nc.sync.dma_start(out=xt_sb[1][:], in_=xt_d[128:256, :])
            wp_sb = wts.tile([128, 2048], BF16, name="wp_sb")
            nc.sync.dma_start(out=wp_sb[:], in_=wp_d[:])
            nc.sync.dma_start(out=xt_sb[2][:], in_=xt_d[256:384, :])
            nc.sync.dma_start(out=xt_sb[3][:], in_=xt_d[384:512, :])

            # ---- PE warmup: the cost model ramps the PE to full clock only
            # after ~3us of continuous busy. Burn f32 matmuls on a memset
            # tile during the DMA lead-in so real work starts at full speed.
            psW = psA.tile([128, 128], F32, name="ps_w", tag="psA")
            for r in range(N_WARMUP):
                nc.tensor.matmul(psW[:], wu[:], wu[:],
                                 start=(r == 0), stop=(r == N_WARMUP - 1))

            # ---- constants ----
            # gpsimd can't write bf16; build f32 then DVE copy-cast.
            ident32 = const.tile([128, 128], F32, name="ident32")
            make_identity(nc, ident32)
            ident = const.tile([128, 128], BF16, name="ident")
            nc.vector.tensor_copy(ident[:], ident32[:])
            # maskT[a, b] = 0 where a >= b else MASK_VAL; used as lhsT so the
            # psum receives M[j, i] = maskT[i, j] = 0 iff i >= j.
            maskf32 = const.tile([128, 128], F32, name="maskf32")
            nc.gpsimd.memset(maskf32[:], 0.0)
            nc.gpsimd.affine_select(
                out=maskf32[:], in_=maskf32[:],
                compare_op=mybir.AluOpType.is_ge, fill=MASK_VAL,
                base=0, channel_multiplier=1, pattern=[[-1, 128]],
            )
            maskT = const.tile([128, 128], BF16, name="maskT")
            nc.vector.tensor_copy(maskT[:], maskf32[:])
            ones32 = const.tile([1, 128], F32, name="ones32")
            nc.gpsimd.memset(ones32[:], 1.0)
            ones_b = const.tile([1, 128], BF16, name="ones_b")
            nc.vector.tensor_copy(ones_b[:], ones32[:])

            # ---- fp8 pair access-pattern helpers ----
            PCOL = {0: 0, 2: 1, 1: 2, 3: 3}   # p -> column block in wqk

            def wqk_main(p, kp):
                # (w_hi[2kp], w_hi[2kp+1])  [128, 2, 128]
                off = 4096 + PCOL[p] * 1024 + (2 * kp) * 128
                return wqk_sb[:, off:off + 256].rearrange(
                    "a (two m) -> a two m", two=2)

            def wqk_cross(p, kt):
                # (w_lo[kt], w_hi[kt])  dim1 stride 4096; pairs with
                # xt_cross's (x_hi, x_lo) -> lo*hi + hi*lo
                return wqk_sb[:].rearrange(
                    "a (h pkm) -> a h pkm", h=2)[
                        :, :, PCOL[p] * 1024 + kt * 128:
                        PCOL[p] * 1024 + (kt + 1) * 128]

            def xt_main(Q, kp, lo=0, n=512):
                # (x_hi[2kp], x_hi[2kp+1])  [128, 2, n]
                return xt_sb[Q][:].rearrange(
                    "a (h kt m) -> a h kt m", h=2, kt=8)[
                        :, 0, 2 * kp:2 * kp + 2, lo:lo + n]

            def xt_cross(Q, kt, lo=0, n=512):
                # (x_hi[kt], x_lo[kt])  dim1 stride 4096
                return xt_sb[Q][:].rearrange(
                    "a (h km) -> a h km", h=2)[
                        :, :, kt * 512 + lo:kt * 512 + lo + n]

            def wv_main(kp):
                off = 2080 + (2 * kp) * 260
                return wv_sb[:, off:off + 520].rearrange(
                    "a (two m) -> a two m", two=2)

            def wv_cross(kt):
                # (wv_lo[kt], wv_hi[kt])  dim1 stride 2080
                return wv_sb[:].rearrange(
                    "a (h kn) -> a h kn", h=2)[:, :, kt * 260:(kt + 1) * 260]

            # ---- persistent activations ----
            qkT = [qkt.tile([128, T], BF16, name=f"qkT{p}", tag=f"qkT{p}")
                   for p in range(4)]
            v_sb = [vsb.tile([128, 260], BF16, name=f"v{jt}", tag=f"v{jt}")
                    for jt in range(NT)]

            # ---------------- emission units ----------------
            # filler units: (est_pe_ns, closure). Emitted between attention
            # stages to keep PE busy while ACT chews exp.
            def b_group(Q, p):
                # qkT[p, Q] = (w_hi+w_lo).T (x_hi+x_lo) - w_lo.T x_lo over
                # K=1024: 4 DoubleRow mains (kt pairs, hi*hi) + 8 crosses.
                def emit():
                    ps = psA.tile([128, 512], F32, name="ps_qk", tag="psA")
                    for kp in range(4):
                        nc.tensor.matmul(
                            ps[:], wqk_main(p, kp), xt_main(Q, kp),
                            start=(kp == 0), stop=False, perf_mode=DR)
                    for kt in range(NKT):
                        nc.tensor.matmul(
                            ps[:], wqk_cross(p, kt), xt_cross(Q, kt),
                            start=False, stop=(kt == NKT - 1), perf_mode=DR)
                    nc.vector.tensor_scalar_add(
                        qkT[p][:, Q * 512:(Q + 1) * 512], ps[:],
                        bqk_sb[:, p:p + 1])
                return (DRB_NS, emit)

            def c_group(Q, mtl):
                def emit():
                    jt = 4 * Q + mtl
                    ps = psA.tile([128, 260], F32, name="ps_v", tag="psA")
                    for kp in range(4):
                        nc.tensor.matmul(
                            ps[:], xt_main(Q, kp, lo=mtl * 128, n=128),
                            wv_main(kp),
                            start=(kp == 0), stop=False, perf_mode=DR)
                    for kt in range(NKT):
                        nc.tensor.matmul(
                            ps[:], xt_cross(Q, kt, lo=mtl * 128, n=128),
                            wv_cross(kt),
                            start=False, stop=False, perf_mode=DR)
                    nc.tensor.matmul(ps[:], ones_b[:], wvl_sb[:],
                                     start=False, stop=True)
                    nc.vector.tensor_copy(v_sb[jt][:], ps[:])
                return (DRC_NS, emit)

            y_norm = {}   # (ic, itl) -> tile
            yT = {}       # (ic, kt) -> tile

            def tre_group(ic, mtl, tail=False):
                """Transpose y_norm[ic, mtl] into yT and project+store.
                yT[(ic,)] is one [128, 1024] tile: kt block at col kt*512.
                tail=True spreads evictions across DVE and ACT (end of
                kernel, ACT is idle)."""
                def emit():
                    mt = 4 * ic + mtl
                    pst = psA.tile([128, 256], BF16, name="ps_tr", tag="psA")
                    for kt in range(2):
                        nc.tensor.transpose(
                            pst[:, kt * 128:(kt + 1) * 128],
                            y_norm[(ic, mtl)][:, kt * 128:(kt + 1) * 128],
                            ident[:])
                    # one strided evict writes both kt blocks of yT
                    dst = yT[(ic,)][:].rearrange(
                        "p (a b) -> p a b", a=2)[:, :, mtl * 128:(mtl + 1) * 128]
                    if tail:
                        nc.scalar.copy(dst, pst[:])
                    else:
                        nc.vector.tensor_copy(dst, pst[:])
                    o = outp.tile([128, 1024], BF16, name="o_t", tag="o_t")
                    for nch in range(2):
                        ps = psA.tile([128, 512], F32, name="ps_o", tag="psA")
                        for kt in range(2):
                            nc.tensor.matmul(
                                ps[:],
                                yT[(ic,)][:, kt * 512 + mtl * 128:
                                          kt * 512 + (mtl + 1) * 128],
                                wp_sb[:, kt * 1024 + nch * 512: kt * 1024 + (nch + 1) * 512],
                                start=(kt == 0), stop=(kt == 1),
                            )
                        if tail and nch == 0:
                            nc.scalar.copy(o[:, nch * 512:(nch + 1) * 512], ps[:])
                        else:
                            nc.vector.tensor_copy(
                                o[:, nch * 512:(nch + 1) * 512], ps[:])
                    # one store per m-tile: each DMA costs ~650ns of
                    # serialized SP/HWDGE issue regardless of size
                    nc.sync.dma_start(
                        out=out_d[mt * 128:(mt + 1) * 128, :], in_=o[:])
                return (2 * 128 * 0.4167 + 4 * MM_NS, emit)

            def itl_key(mtl):
                return mtl

            # global filler deque: (pe_ns, emit, marker). markers order
            # dependencies: ("B", ic) must emit before attn(ic)'s att reads
            # qkT; ("C", ic) before attn(ic)'s diagonal AVs read v.
            filler = []
            consumed = {("B", 0)}
            # global emission clock (ns estimates): pe = PE busy frontier,
            # act = ACT (exp) completion frontier. Used to decide when PE
            # needs filler so it never idles waiting for exp.
            clk = {"pe": 0.0, "act": 0.0}
            SEM_LAT = 100.0
            tail_mode = [False]

            def pop_one():
                pe_ns, emit, marker, ready = filler.pop(0)
                emit()
                if marker:
                    consumed.add(marker)
                clk["pe"] += pe_ns

            def pop_filler(need_pe_ns, force=False):
                got = 0.0
                while filler and got < need_pe_ns:
                    if not force and filler[0][3] > clk["pe"]:
                        break  # head unit's inputs not DMA'd yet
                    pe_ns = filler[0][0]
                    pop_one()
                    got += pe_ns
                return got

            def advance_pe_to(t):
                """PE must reach estimated time t before the next emitted
                instruction can run: emit filler to cover the wait."""
                while clk["pe"] < t and filler and filler[0][3] <= clk["pe"]:
                    pop_one()
                if clk["pe"] < t:
                    clk["pe"] = t  # PE idles

            def force_pop(marker):
                while marker not in consumed and filler:
                    pop_one()

            # ---------------- attention head ----------------
            LAG = 3

            def atthead(s, ic, prev_fin=None, prev_fin_ready=0.0):
                if ic == 0 and s == 2:
                    # heads 2,3 read qkT p1/p3, which ride the filler
                    force_pop(("B0hi", 0))
                qrow = (s % 2) * 64
                qtile = qkT[s // 2]
                ktile = qkT[2 + s // 2]
                ps_y = psY.tile([128, 260], F32, name="ps_y", tag="psY")
                n_av = [0] * 4            # AVs emitted per i-tile region
                tot_av = [4 * ic + itl + 1 for itl in range(4)]
                # jt -> (e tile, col of i-block 0); filled as stages emit
                e_of = {}

                def av(itl, jt):
                    # PSUM groups are bank(2KB)-granular: regions of ps_y
                    # accumulate strictly one group at a time (r0 rolls with
                    # the stages; r1..r3 burst after all e tiles exist).
                    if jt >= 4 * ic:
                        force_pop(("C", ic))
                    e, base = e_of[jt]
                    nc.tensor.matmul(
                        ps_y[:, itl * 65:itl * 65 + 65],
                        e[:, base + itl * 128: base + itl * 128 + 128],
                        v_sb[jt][:, s * 65:s * 65 + 65],
                        start=(n_av[itl] == 0),
                        stop=(n_av[itl] == tot_av[itl] - 1),
                    )
                    n_av[itl] += 1

                # stage list: full pairs first, then the diagonal halves.
                att_stages = []

                for pr in range(2 * ic):
                    def mk_att(pr=pr):
                        ps_a = psE.tile([128, 1024], F32, name="ps_a", tag="psE")
                        e = ep.tile([128, 1024], BF16, name="e_t", tag="e_t")
                        for h in range(2):
                            jt = 2 * pr + h
                            nc.tensor.matmul(
                                ps_a[:, h * 512:(h + 1) * 512],
                                ktile[qrow:qrow + 64, jt * 128:(jt + 1) * 128],
                                qtile[qrow:qrow + 64, ic * 512:(ic + 1) * 512],
                                start=True, stop=True,
                            )
                            e_of[jt] = (e, h * 512)
                        nc.scalar.activation(
                            e[:], ps_a[:], mybir.ActivationFunctionType.Exp,
                            scale=float(HD) ** -0.5 / (WSCALE * WSCALE))
                    # stage jts, ACT ns, PE ns
                    att_stages.append((mk_att, [2 * pr, 2 * pr + 1],
                                       EXP_PAIR_NS, 2 * MM_NS))

                for half in range(2):
                    def mk_att(half=half):
                        ps_a = psE.tile([128, 1024], F32, name="ps_a", tag="psE")
                        e = ep.tile([128, 1024], BF16, name="e_t", tag="e_t")
                        for h in range(2):
                            o = (2 * half + h) * 128
                            jt = 4 * ic + 2 * half + h
                            # diagonal 128-col block: att + mask accumulated
                            nc.tensor.matmul(
                                ps_a[:, h * 512 + o: h * 512 + o + 128],
                                ktile[qrow:qrow + 64, jt * 128:(jt + 1) * 128],
                                qtile[qrow:qrow + 64,
                                      ic * 512 + o: ic * 512 + o + 128],
                                start=True, stop=False,
                            )
                            nc.tensor.matmul(
                                ps_a[:, h * 512 + o: h * 512 + o + 128],
                                maskT[:], ident[:],
                                start=False, stop=True,
                            )
                            # unmasked remainder of the row band
                            if o + 128 < 512:
                                nc.tensor.matmul(
                                    ps_a[:, h * 512 + o + 128:(h + 1) * 512],
                                    ktile[qrow:qrow + 64, jt * 128:(jt + 1) * 128],
                                    qtile[qrow:qrow + 64,
                                          ic * 512 + o + 128:(ic + 1) * 512],
                                    start=True, stop=True,
                                )
                            e_of[jt] = (e, h * 512)
                            nc.scalar.activation(
                                e[:, h * 512 + o:(h + 1) * 512],
                                ps_a[:, h * 512 + o:(h + 1) * 512],
                                mybir.ActivationFunctionType.Exp,
                                scale=float(HD) ** -0.5 / (WSCALE * WSCALE))
                    jts = [4 * ic + 2 * half, 4 * ic + 2 * half + 1]
                    att_pe = (896 - 512 * half) * 0.4167 + 2 * 128 * 0.4167
                    exp_ns = (896 - 512 * half) * 0.833 + 2 * EXP_INIT_NS
                    att_stages.append((mk_att, jts, exp_ns, att_pe))

                # emit: stages; region 0's group rolls along (lagged);
                # regions 1..3 burst after the last stage. The global clock
                # models the psE ring (bufs=2): att stage k reuses stage
                # k-2's psum, so PE must not reach att(k) before exp(k-2)
                # completes -- filler covers the difference.
                AV_NS = 65 * 0.4167
                done = []       # per stage: jts whose e exists
                exp_done = []   # per stage: est. exp completion time
                fin_emitted = [prev_fin is None]
                for k, (mk, jts, exp_ns, att_pe) in enumerate(att_stages):
                    sid = len(stage_registry)
                    stage_registry.append((sid, nc.next_id()))
                    h = hints.get(sid, 0.0)
                    if h > 0:
                        pop_filler(h, force=True)
                    if k >= 2:
                        advance_pe_to(exp_done[k - 2])
                    mk()
                    clk["pe"] += att_pe
                    clk["act"] = max(clk["act"],
                                     clk["pe"] + SEM_LAT) + exp_ns
                    exp_done.append(clk["act"] + SEM_LAT)
                    done.append(jts)
                    if k >= LAG:
                        for jt in done[k - LAG]:
                            if jt <= 4 * ic:
                                av(0, jt)
                                clk["pe"] += AV_NS
                    # previous head's deferred tail: emit once its exps are
                    # surely done (covered by our att stream, not filler)
                    if not fin_emitted[0] and (
                            clk["pe"] >= prev_fin_ready
                            or k >= len(att_stages) - 2):
                        prev_fin()
                        fin_emitted[0] = True
                if not fin_emitted[0]:
                    advance_pe_to(prev_fin_ready)
                    prev_fin()

                def fin():
                    # tail: r0 leftovers + r1 only need the half0 diag exp;
                    # r2/r3 additionally need half1 -- gate separately so
                    # the early regions stream while half1's exp finishes.
                    if len(exp_done) >= 2:
                        advance_pe_to(exp_done[-2])
                    for jts in done[max(0, len(done) - LAG):]:
                        for jt in jts:
                            if jt <= 4 * ic:
                                av(0, jt)
                                clk["pe"] += AV_NS
                    for jt in range(4 * ic + 2):
                        av(1, jt)
                        clk["pe"] += AV_NS
                    # ps_y groups are bank-granular: one region open at a
                    # time. Only r2/r3's diagonal jts need the half1 exp, so
                    # stream r2's earlier jts first, wait, close r2, then r3.
                    for jt in range(4 * ic + 2):
                        av(2, jt)
                        clk["pe"] += AV_NS
                    if exp_done:
                        advance_pe_to(exp_done[-1])
                    for jt in range(4 * ic + 2, 4 * ic + 3):
                        av(2, jt)
                        clk["pe"] += AV_NS
                    for jt in range(4 * ic + 4):
                        av(3, jt)
                        clk["pe"] += AV_NS

                    # normalize: rs = 1/S per i-tile, per-partition scalars
                    rs = rsp.tile([128, 4], F32, name="rs_t", tag="rs_t")
                    with nc.allow_low_precision(
                            reason="softmax sum reciprocal"):
                        nc.vector.reciprocal(rs[:], ps_y[:, 64::65])
                    for itl in range(4):
                        yn = y_norm.get((ic, itl))
                        if yn is None:
                            yn = ynp.tile([128, 256], BF16, name="y_n",
                                          tag=f"yn{(ic % 2) * 4 + itl}")
                            y_norm[(ic, itl)] = yn
                        if tail_mode[0] and itl % 2 == 1:
                            # end of kernel: ACT is idle, split the norms
                            nc.scalar.mul(
                                yn[:, s * 64:(s + 1) * 64],
                                ps_y[:, itl * 65:itl * 65 + 64],
                                rs[:, itl:itl + 1])
                        else:
                            nc.vector.tensor_scalar(
                                out=yn[:, s * 64:(s + 1) * 64],
                                in0=ps_y[:, itl * 65:itl * 65 + 64],
                                scalar1=rs[:, itl:itl + 1],
                                scalar2=None,
                                op0=mybir.AluOpType.mult,
                            )
                return fin, (exp_done[-2] if len(exp_done) >= 2 else 0.0)

            # ---------------- schedule ----------------
            def attn_quarter(ic):
                force_pop(("B", ic))
                yT[(ic,)] = ytp.tile([128, 1024], BF16, name="yT_t",
                                     tag=f"yT{ic % 2}")
                fin, ready = None, 0.0
                for s in range(4):
                    fin, ready = atthead(s, ic, fin, ready)
                    if ic == 3 and s == 2:
                        # fin(s=1) was just emitted inside atthead(2).
                        # yT's kt=0 half only reads heads 0,1 (cols 0..127
                        # of y_norm): transpose it now so it fills the last
                        # head's exp window and shortens the final tail.
                        for m in range(4):
                            pst = psA.tile([128, 128], BF16,
                                           name="ps_tr", tag="psA")
                            nc.tensor.transpose(
                                pst[:], y_norm[(3, m)][:, 0:128], ident[:])
                            nc.vector.tensor_copy(
                                yT[(3,)][:, m * 128:(m + 1) * 128], pst[:])
                        clk["pe"] += 4 * 128 * 0.4167
                # last head's tail (fin gates its own exp waits with filler)
                if ic == 3:
                    tail_mode[0] = True
                fin()

            # it0: B(0) p0/p2 staged by kt-halves so PE starts on the first
            # DMA arrivals. Heads 0,1 of attn(0) only need p0 (their q) and
            # p2 (their k), so attention -- and with it ACT's exp stream --
            # can start ~5us earlier; B(0) p1/p3 and C(0) ride the filler.
            # hi*hi mains first (only need the hi DMA planes), then the
            # cross terms once the lo planes land.
            psB0 = {}
            for p in (0, 2):
                psB0[p] = psA.tile([128, 512], F32, name="ps_qk", tag="psA")
                for kp in range(4):
                    nc.tensor.matmul(psB0[p][:], wqk_main(p, kp),
                                     xt_main(0, kp),
                                     start=(kp == 0), stop=False, perf_mode=DR)
            for p in (0, 2):
                for kt in range(NKT):
                    nc.tensor.matmul(
                        psB0[p][:], wqk_cross(p, kt), xt_cross(0, kt),
                        start=False, stop=(kt == NKT - 1), perf_mode=DR)
                nc.vector.tensor_scalar_add(
                    qkT[p][:, :512], psB0[p][:], bqk_sb[:, p:p + 1])
            # startup estimate: DMA lead-in + warmup + B(0) p0/p2
            clk["pe"] = 6000.0
            filler.append(b_group(0, 1) + (None, 9000.0))
            filler.append(b_group(0, 3) + (("B0hi", 0), 9700.0))
            for mtl in range(4):
                filler.append(c_group(0, mtl) + (
                    (("C", 0),) if mtl == 3 else (None,)) + (8300.0,))

            # everything else rides the filler queue, FIFO-ordered so
            # earlier-needed work is popped first.
            # ready = rough DMA arrival estimate of xt[Q] (ns)
            XT_READY = xt_ready or {1: 12700.0, 2: 17100.0, 3: 20000.0}
            for Q in range(1, NQ):
                rdy = XT_READY[Q]
                filler.append(b_group(Q, 0) + (None, rdy))
                filler.append(b_group(Q, 1) + (None, rdy))
                filler.append(b_group(Q, 2) + (None, rdy))
                filler.append(b_group(Q, 3) + (("B", Q), rdy))
                filler.append(c_group(Q, 0) + (None, rdy))
                filler.append(c_group(Q, 1) + (None, rdy))
                filler.append(c_group(Q, 2) + (None, rdy))
                filler.append(c_group(Q, 3) + (("C", Q), rdy))

            clk_marks = {"attn0_start": clk["pe"]}
            attn_quarter(0)
            clk_marks["attn0_end"] = clk["pe"]
            filler += [tre_group(0, m) + ((("T", 0) if m == 3 else None),
                                          clk["pe"] + tre_ready[0]) for m in range(4)]
            attn_quarter(1)
            clk_marks["attn1_end"] = clk["pe"]
            filler += [tre_group(1, m) + ((("T", 1) if m == 3 else None),
                                          clk["pe"] + tre_ready[1]) for m in range(4)]
            attn_quarter(2)
            clk_marks["attn2_end"] = clk["pe"]
            filler += [tre_group(2, m) + (None, clk["pe"] + tre_ready[2]) for m in range(4)]
            attn_quarter(3)
            clk_marks["attn3_end"] = clk["pe"]
            while filler:   # hard drain: ready-gating no longer applies
                pop_one()
            # batched final TRE: the kt=1 transposes+evicts first (their
            # norm waits pipeline on DVE/ACT), then all projections+stores.
            for m in range(4):
                pst = psA.tile([128, 128], BF16, name="ps_tr", tag="psA")
                nc.tensor.transpose(
                    pst[:], y_norm[(3, m)][:, 128:256], ident[:])
                dst = yT[(3,)][:, 512 + m * 128: 512 + (m + 1) * 128]
                if m % 2 == 0:
                    nc.scalar.copy(dst, pst[:])
                else:
                    nc.vector.tensor_copy(dst, pst[:])
            for m in range(4):
                mt = 12 + m
                o = outp.tile([128, 1024], BF16, name="o_t", tag="o_t")
                # exp is done -- reuse the (2-bank) psE tiles so the last
                # four projections pipeline instead of serializing on psA
                ps = psE.tile([128, 1024], F32, name="ps_a", tag="psE")
                for nch in range(2):
                    for kt in range(2):
                        nc.tensor.matmul(
                            ps[:, nch * 512:(nch + 1) * 512],
                            yT[(3,)][:, kt * 512 + m * 128:
                                     kt * 512 + (m + 1) * 128],
                            wp_sb[:, kt * 1024 + nch * 512:
                                  kt * 1024 + (nch + 1) * 512],
                            start=(kt == 0), stop=(kt == 1),
                        )
                    if nch == 0:
                        nc.scalar.copy(o[:, nch * 512:(nch + 1) * 512],
                                       ps[:, nch * 512:(nch + 1) * 512])
                    else:
                        nc.vector.tensor_copy(
                            o[:, nch * 512:(nch + 1) * 512],
                            ps[:, nch * 512:(nch + 1) * 512])
                    if m == 3:
                        # last tile: stream each half as soon as it lands
                        nc.sync.dma_start(
                            out=out_d[mt * 128:(mt + 1) * 128,
                                      nch * 512:(nch + 1) * 512],
                            in_=o[:, nch * 512:(nch + 1) * 512])
                if m != 3:
                    nc.sync.dma_start(
                        out=out_d[mt * 128:(mt + 1) * 128, :], in_=o[:])

    if split_waits:
        _split_matmul_waits(nc)
    nc._stage_registry = stage_registry
    nc._clk_marks = clk_marks
    return nc


def _split_matmul_waits(nc):
    """Walrus codegen in this pipeline allows only one sync wait per
    instruction for most ISA structs (S3_LW, PSEUDO_DMA_DIRECT2D, S3D3_TS,
    ...). Move extra waits onto inserted NoOps on the same engine (program
    order preserves semantics)."""
    n_split = 0
    for bb in nc.main_func.blocks:
        out = []
        for ins in bb.instructions:
            si = getattr(ins, "sync_info", None)
            if (si is not None and len(si.on_wait) >= 2
                    and type(ins).__name__ != "InstNoOp"):
                for w in si.on_wait[:-1]:
                    nop = mybir.InstNoOp(name=f"I-wsplit-{nc.next_id()}",
                                         ins=[], outs=[])
                    nop.engine = ins.engine
                    nop.sync_info = mybir.SyncInfo(on_wait=[w], on_update=[])
                    out.append(nop)
                    n_split += 1
                ins.sync_info = mybir.SyncInfo(
                    on_wait=[si.on_wait[-1]], on_update=si.on_update)
            out.append(ins)
        bb.instructions[:] = out
    return n_split


def _bf16(a):
    import ml_dtypes
    return np.ascontiguousarray(a.astype(ml_dtypes.bfloat16))


def _fp8_hl(a):
    """Split f32 array into (hi, lo) fp8e4m3 planes with hi+lo ~= a."""
    import ml_dtypes
    hi = a.astype(ml_dtypes.float8_e4m3)
    lo = (a - hi.astype(np.float32)).astype(ml_dtypes.float8_e4m3)
    return hi, lo


def shard_inputs(x, Wqkv, bqkv, Wproj, bproj):
    x = np.asarray(x, np.float32)
    Wqkv = np.asarray(Wqkv, np.float32)
    bqkv = np.asarray(bqkv, np.float32)
    Wproj = np.asarray(Wproj, np.float32)
    in_maps = []
    xt_b = []
    import ml_dtypes
    for b in range(B):
        # xt[Q*128+p, half*4096 + kt*512 + m] = hl(x[b][Q*512+m, kt*128+p])
        xT = x[b].T  # [C, T]
        xt = xT.reshape(NKT, 128, NQ, 512).transpose(2, 1, 0, 3).reshape(
            NQ * 128, 4096)
        hi, lo = _fp8_hl(xt)
        xt2 = np.concatenate(
            [hi.reshape(NQ * 128, 4096), lo.reshape(NQ * 128, 4096)], axis=1
        ).reshape(NQ * 128, 2, 4096)
        # rows are Q-tiles of 128; halves interleave per Q-tile row block:
        # [Q*128+p, half*4096 + km] already correct since reshape keeps rows.
        xt_b.append(np.ascontiguousarray(xt2.reshape(NQ * 128, 8192)))
    for c in range(N_CORES):
        b, hg = c // 4, c % 4
        wqk = np.concatenate(
            [Wqkv[:, hg * 256:(hg + 1) * 256],
             Wqkv[:, C + hg * 256: C + (hg + 1) * 256]], axis=1) * WSCALE
        # [128, half*4096 + pc*1024 + kt*128 + m], half 0=lo 1=hi,
        # p stored in order (0,2,1,3)
        wqk2 = wqk.reshape(NKT, 128, 4, 128).transpose(1, 2, 0, 3)[
            :, [0, 2, 1, 3]].reshape(128, 4096)
        hi, lo = _fp8_hl(wqk2)
        wqk3 = np.concatenate([lo, hi], axis=1)
        bqk = np.concatenate(
            [bqkv[hg * 256:(hg + 1) * 256],
             bqkv[C + hg * 256: C + (hg + 1) * 256]]) * WSCALE  # [512]
        bqk2 = np.ascontiguousarray(bqk.reshape(4, 128).T)  # [128, 4]
        wv = np.zeros((C, 260), np.float32)
        wvl = np.zeros((1, 260), np.float32)
        for s in range(4):
            h = 4 * hg + s
            wv[:, s * 65:s * 65 + 64] = Wqkv[:, 2 * C + h * 64: 2 * C + (h + 1) * 64]
            wvl[0, s * 65:s * 65 + 64] = bqkv[2 * C + h * 64: 2 * C + (h + 1) * 64]
            wvl[0, s * 65 + 64] = 1.0
        wv *= WSCALE
        wvl *= WSCALE
        wv2 = wv.reshape(NKT, 128, 260).transpose(1, 0, 2).reshape(128, 8 * 260)
        hi, lo = _fp8_hl(wv2)
        # [128, half*2080 + kt*260 + n], half 0=lo 1=hi
        wv3 = np.concatenate(
            [lo.reshape(128, 2080), hi.reshape(128, 2080)], axis=1)
        wp = Wproj[hg * 256:(hg + 1) * 256, :]  # [256, C]
        wp2 = wp.reshape(2, 128, C).transpose(1, 0, 2).reshape(128, 2048)
        in_maps.append({
            "xt": xt_b[b],
            "wqk": np.ascontiguousarray(wqk3),
            "bqk": np.ascontiguousarray(bqk2, dtype=np.float32),
            "wv": np.ascontiguousarray(wv3),
            "wvl": _bf16(wvl),
            "wp": _bf16(wp2),
        })
    return in_maps


_NC_CACHE = {}


def kernel(x, Wqkv, bqkv, Wproj, bproj):
    from concourse.bass_utils import run_bass_kernel_spmd

    if "nc" not in _NC_CACHE:
        _NC_CACHE["nc"] = build_nc()
    nc = _NC_CACHE["nc"]
    in_maps = shard_inputs(x, Wqkv, bqkv, Wproj, bproj)
    res = run_bass_kernel_spmd(nc, in_maps, list(range(N_CORES)))
    _NC_CACHE["last_exec_time_ns"] = res.exec_time_ns
    bproj = np.asarray(bproj, np.float32)
    out = np.zeros((B, T, C), np.float32)
    for c in range(N_CORES):
        out[c // 4] += np.asarray(res.results[c]["out"], np.float32)
    out += bproj[None, None, :]
    return out

